# revision 27
# baseline (speedup 1.0000x reference)
"""DMSA (dual-modal channel cross-attention) Trainium2 kernel — v5.

Sharding: 8 cores = 2 batches x 4 bands of 32 image rows. Each core
computes its band fully; the channel attention's per-head Gram matrices
(contraction over all n = h*w tokens, with l2-normalization folded in
via the Gram diagonal) are summed with one AllReduce per 4-core group.

Device layout: channel-major activations [128 partitions, 2 channel
halves, tokens]. Stage-1 runs on an unpadded 36x128 ext-row grid
(9 tiles x 512 tokens); v is spilled in fp16 to a width-padded 36x130
DRAM grid (pad columns kept zero inside the spill tile) so both 3x3
depthwise convs read taps as plain offset views.

Engines: big matmuls f32r; q/k path bf16; v/conv path fp16. conv1 runs
on DVE in half-chunk thunks drained one per image row so PSUM evicts
never queue behind it. conv2 runs as 9 diagonal PE matmuls per output
group, evicted to SBUF (bias folded) so ten groups precede the softmax
on the PE queue and cover the AllReduce window; the projection pass
then adds on top via DVE. Weights arrive in 5 blob DMAs.
"""
import numpy as np
import ml_dtypes
from contextlib import ExitStack

import concourse.bass as bass
import concourse.tile as tile
import concourse.mybir as mybir
from concourse import bacc
from concourse.bass_utils import run_bass_kernel_spmd

F32 = mybir.dt.float32
F32R = mybir.dt.float32r
BF16 = mybir.dt.bfloat16
F16 = mybir.dt.float16
AF = mybir.ActivationFunctionType
OP = mybir.AluOpType

B, H, W, C = 2, 128, 128, 256
HEADS, DH = 8, 32
RB = 32             # image rows per core
ER = RB + 4         # ext rows
WP = W + 2          # padded width (conv grid)
GN = ER * WP        # padded tokens (v spill grid) = 4680
EN = ER * W         # unpadded ext tokens (stage-1 grid) = 4608
NV = RB * W         # valid tokens = 4096
NT = 9              # stage-1 tiles (4 ext rows each)
LRELU_A = 0.01
# conv1 chunk g-row ranges and the stage-1 tile after which each may run
C1CHUNKS = [(0, 6, 1), (6, 12, 3), (12, 18, 4), (18, 24, 6), (24, 30, 7),
            (30, 34, None)]  # None -> after the collective

# weight blobs: name -> (group, shape); group s = hot f32r (tile-0 deps)
WSPEC = {
    "fxw1T": ("s", [128, 4, 2, 128]), "fyw1T": ("s", [128, 4, 2, 128]),
    "qw1T": ("s", [128, 2, 2, 128]), "vw1T": ("s", [128, 2, 2, 128]),
    "vw2T": ("r", [128, 2, 2, 128]), "pxwT": ("r", [128, 2, 256]),
    "pywT": ("r", [128, 2, 256]), "blk128": ("r", [128, 128]),
    "eye32r": ("f", [128, 32]), "eye8": ("f", [128, 8, 32]),
    "bfx": ("f", [128, 2]), "bfy": ("f", [128, 2]), "bq": ("f", [128, 2]),
    "bkx": ("f", [128, 2]), "bky": ("f", [128, 2]), "bv": ("f", [128, 2]),
    "obx": ("f", [128, 2]), "oby": ("f", [128, 2]), "b1c": ("f", [128, 2]),
    "rxy_exp": ("f", [128, 4]), "w1c": ("f", [128, 2, 9]),
    "qw2T": ("b", [128, 2, 256]), "kw2T": ("b", [128, 2, 256]),
    "kxw1T": ("b", [128, 2, 2, 128]), "kyw1T": ("b", [128, 2, 2, 128]),
    "dw2": ("h", [128, 2, 9, 128]), "eye128": ("h", [128, 128]),
}
GDT = {"s": F32R, "r": F32R, "f": F32, "b": BF16, "h": F16}

_CACHED = {}


def _nc_build():
    nc = bacc.Bacc(num_devices=8)

    gsz = {g: 0 for g in GDT}
    woff = {}
    for name, (g, shp) in WSPEC.items():
        n = int(np.prod(shp[1:]))
        woff[name] = gsz[g]
        gsz[g] += n

    din = {}
    for g in GDT:
        din[g] = nc.dram_tensor(f"wg_{g}", [128, gsz[g]], GDT[g],
                                kind="ExternalInput")
    xin = nc.dram_tensor("xin", [128, 2, EN], F32R, kind="ExternalInput")
    yin = nc.dram_tensor("yin", [128, 2, EN], F32R, kind="ExternalInput")
    gm0_t = nc.dram_tensor("gm0", [128, 1], F32, kind="ExternalInput")
    gm33_t = nc.dram_tensor("gm33", [128, 1], F32, kind="ExternalInput")

    out_x = nc.dram_tensor("out_x", [128, 2, NV], F16, kind="ExternalOutput")
    out_y = nc.dram_tensor("out_y", [128, 2, NV], F16, kind="ExternalOutput")
    vsp_x = nc.dram_tensor("vsp_x", [128, 2, GN], F16, kind="Internal")
    vsp_y = nc.dram_tensor("vsp_y", [128, 2, GN], F16, kind="Internal")
    cc_in = nc.dram_tensor("cc_in", [128, HEADS, 2, 32], F32,
                           kind="Internal")
    cc_out = nc.dram_tensor("cc_out", [128, HEADS, 2, 32], F32,
                            kind="Internal")

    with tile.TileContext(nc) as tc, ExitStack() as ctx:
        wp = ctx.enter_context(tc.tile_pool(name="wp", bufs=1))
        io = ctx.enter_context(tc.tile_pool(name="io", bufs=2))
        hidF = ctx.enter_context(tc.tile_pool(name="hidF", bufs=2))
        hidQ = ctx.enter_context(tc.tile_pool(name="hidQ", bufs=2))
        hidV = ctx.enter_context(tc.tile_pool(name="hidV", bufs=2))
        stk = ctx.enter_context(tc.tile_pool(name="stk", bufs=2))
        sm = ctx.enter_context(tc.tile_pool(name="sm", bufs=1))
        gb = ctx.enter_context(tc.tile_pool(name="gb", bufs=1))
        cvp = ctx.enter_context(tc.tile_pool(name="cvp", bufs=2))
        ot = ctx.enter_context(tc.tile_pool(name="ot", bufs=2))
        vtp = ctx.enter_context(tc.tile_pool(name="vtp", bufs=4))
        cp = ctx.enter_context(tc.tile_pool(name="cp", bufs=12))
        psA = ctx.enter_context(tc.tile_pool(name="psA", bufs=4, space="PSUM"))
        psQ = ctx.enter_context(tc.tile_pool(name="psQ", bufs=2, space="PSUM"))
        psG = ctx.enter_context(tc.tile_pool(name="psG", bufs=1, space="PSUM"))

        # hot weights first, then the first input tiles, then other blobs
        w = {}
        wt = {}
        wt["s"] = wp.tile([128, gsz["s"]], GDT["s"], tag="wg_s",
                          name="wg_s")
        nc.sync.dma_start(wt["s"][:], din["s"].ap())
        xt0 = io.tile([128, 2, 512], F32R, tag="xt")
        nc.sync.dma_start(xt0[:], xin.ap()[:, :, 0:512])
        yt0 = io.tile([128, 2, 512], F32R, tag="yt")
        nc.sync.dma_start(yt0[:], yin.ap()[:, :, 0:512])
        for g in GDT:
            if g == "s":
                continue
            wt[g] = wp.tile([128, gsz[g]], GDT[g], tag=f"wg_{g}",
                            name=f"wg_{g}")
            nc.sync.dma_start(wt[g][:], din[g].ap())
        for name, (g, shp) in WSPEC.items():
            v = wt[g][:, woff[name]:woff[name] + int(np.prod(shp[1:]))]
            if len(shp) == 3:
                v = v.rearrange("p (a b) -> p a b", a=shp[1])
            elif len(shp) == 4:
                v = v.rearrange("p (a b c) -> p a b c", a=shp[1], b=shp[2])
            w[name] = v
        for name, t in (("gm0", gm0_t), ("gm33", gm33_t)):
            tl = wp.tile([128, 1], F32, tag=f"w_{name}", name=f"w_{name}")
            nc.sync.dma_start(tl[:], t.ap())
            w[name] = tl

        # warm the activation tables used later so their loads don't land
        # on the softmax critical path
        warm = sm.tile([128, 2], F32, tag="warm")
        nc.vector.memset(warm[:], 1.0)
        wsum = sm.tile([128, 1], F32, tag="wsum")
        nc.scalar.sqrt(warm[:, 0:1], warm[:, 0:1])
        nc.scalar.activation(warm[:, 0:1], warm[:, 0:1], AF.Exp,
                             bias=warm[:, 1:2], accum_out=wsum[:])
        nc.scalar.activation(warm[:, 0:1], warm[:, 0:1], AF.Identity,
                             scale=warm[:, 1:2])
        nc.scalar.activation(warm[:, 0:1], warm[:, 0:1], AF.Identity,
                             bias=warm[:, 1:2])
        nc.scalar.activation(warm[:, 0:1], warm[:, 0:1], AF.Gelu,
                             bias=warm[:, 1:2])
        nc.scalar.activation(warm[:, 0:1], warm[:, 0:1], AF.Lrelu,
                             bias=warm[:, 1:2], alpha=LRELU_A)
        nc.scalar.copy(warm[:, 0:1], warm[:, 1:2])

        gram0 = psG.tile([128, 512], F32, tag="gram0")
        gram1 = psG.tile([128, 512], F32, tag="gram1")
        grams = [gram0, gram1]

        gx = gb.tile([128, 2, ER - 2, WP], F16, tag="gx")
        gy = gb.tile([128, 2, ER - 2, WP], F16, tag="gy")
        nc.scalar.memzero(gx[:])
        nc.scalar.memzero(gy[:])
        TAPS = [(dr, dc) for dr in (-1, 0, 1) for dc in (-1, 0, 1)]

        # pre-zero the pad columns of both rotating spill tiles (their
        # interiors are overwritten each tile; borders stay zero)
        for nm in ("x", "y"):
            for _ in range(2):
                vt = io.tile([128, 2, 4, WP], F16, tag=f"vt{nm}")
                nc.vector.memset(vt[:, :, :, 0:1], 0.0)
                nc.vector.memset(vt[:, :, :, WP - 1:WP], 0.0)

        def conv1_half(vc, gbuf, g0, g1, g):
            """DVE 9-tap fp16 conv1 of one channel half + Gelu evict."""
            vr0 = g0
            nr = g1 - g0
            acc = cvp.tile([128, 6, 128], F16, tag="cacc", name="cacc")
            for i, (dr, dc) in enumerate(TAPS):
                src = vc[:, g, g0 + 1 + dr - vr0:g0 + 1 + dr - vr0 + nr,
                         1 + dc:129 + dc]
                if i == 0:
                    nc.vector.tensor_scalar_mul(acc[:, :nr, :], src,
                                                w["w1c"][:, g, 0:1])
                else:
                    nc.vector.scalar_tensor_tensor(
                        acc[:, :nr, :], src, w["w1c"][:, g, i:i + 1],
                        acc[:, :nr, :], OP.mult, OP.add)
            nc.scalar.activation(gbuf[:, g, g0:g1, 1:129], acc[:, :nr, :],
                                 AF.Gelu, bias=w["b1c"][:, g:g + 1])

        c1q = []

        def push_chunk(gbuf, vsp, g0, g1):
            cell = {}

            def t0():
                vc = cvp.tile([128, 2, 8, WP], F16, tag="vc", name="vc")
                vr1 = min(g1 + 2, ER)
                nc.sync.dma_start(vc[:, :, :vr1 - g0, :],
                                  vsp.ap()[:, :, g0 * WP:vr1 * WP])
                cell["vc"] = vc
                conv1_half(vc, gbuf, g0, g1, 0)

            def t1():
                conv1_half(cell["vc"], gbuf, g0, g1, 1)

            c1q.append(t0)
            c1q.append(t1)

        def drain_one():
            if c1q:
                c1q.pop(0)()

        # ================= stage 1 =================
        vrow = 0
        prev_st = None

        def emit_gram(st, row):
            for h in range(HEADS):
                nc.tensor.matmul(
                    grams[h // 4][:, (h % 4) * 128:(h % 4) * 128 + 128],
                    st[:, h], st[:, h],
                    start=(row == 0), stop=(row == RB - 1),
                    skip_group_check=True)

        def mlp1(srcs, w1T, nk, bias, tag, pool, dt, lo=0, n=512):
            """hidden = lrelu(srcs @ w1T + b); per-half PSUM banks."""
            ht = pool.tile([128, 2, 512], dt, tag=tag)
            for mh in range(2):
                ps = psA.tile([128, 512], F32, tag="psA")
                for k in range(nk):
                    src = srcs[k // 2][:, k % 2, lo:lo + n] if len(srcs) > 1 \
                        else srcs[0][:, k, lo:lo + n]
                    nc.tensor.matmul(ps[:, :n], w1T[:, k, mh, :], src,
                                     start=(k == 0), stop=(k == nk - 1))
                nc.scalar.activation(ht[:, mh, :n], ps[:, :n], AF.Lrelu,
                                     bias=bias[:, mh:mh + 1], alpha=LRELU_A)
            return ht

        for t in range(NT):
            if t == 0:
                xt, yt = xt0, yt0
            else:
                xt = io.tile([128, 2, 512], F32R, tag="xt")
                nc.sync.dma_start(xt[:],
                                  xin.ap()[:, :, t * 512:(t + 1) * 512])
                yt = io.tile([128, 2, 512], F32R, tag="yt")
                nc.sync.dma_start(yt[:],
                                  yin.ap()[:, :, t * 512:(t + 1) * 512])

            # valid-row window within this tile
            e0, e1 = max(2, 4 * t), min(ER - 2, 4 * t + 4)
            lo, n = (e0 - 4 * t) * 128, (e1 - e0) * 128

            fhx = mlp1([xt, yt], w["fxw1T"], 4, w["bfx"], "fhx", hidF, BF16,
                       lo, n)
            fhy = mlp1([xt, yt], w["fyw1T"], 4, w["bfy"], "fhy", hidF, BF16,
                       lo, n)
            qhx = mlp1([xt], w["qw1T"], 2, w["bq"], "qhx", hidQ, BF16, lo, n)
            qhy = mlp1([yt], w["qw1T"], 2, w["bq"], "qhy", hidQ, BF16, lo, n)
            khx = mlp1([fhx], w["kxw1T"], 2, w["bkx"], "khx", hidQ, BF16,
                       0, n)
            khy = mlp1([fhy], w["kyw1T"], 2, w["bky"], "khy", hidQ, BF16,
                       0, n)
            vhx = mlp1([xt], w["vw1T"], 2, w["bv"], "vhx", hidV, F32R)
            vhy = mlp1([yt], w["vw1T"], 2, w["bv"], "vhy", hidV, F32R)

            # v = vhid @ vw2T (ext tokens), fp16 spill to padded DRAM grid
            for nm, vh, vsp in (("x", vhx, vsp_x), ("y", vhy, vsp_y)):
                vt = io.tile([128, 2, 4, WP], F16, tag=f"vt{nm}")
                for g in range(2):
                    ps = psA.tile([128, 512], F32, tag="psA")
                    for k in range(2):
                        nc.tensor.matmul(ps[:], w["vw2T"][:, k, g, :],
                                         vh[:, k, :], start=(k == 0),
                                         stop=(k == 1))
                    nc.vector.tensor_copy(
                        vt[:, g, :, 1:129],
                        ps.rearrange("p (r c) -> p r c", c=128))
                nc.sync.dma_start(
                    vsp.ap().rearrange("p a (r c) -> p a r c", c=WP)
                    [:, :, 4 * t:4 * t + 4, :],
                    vt[:])

            # token-major QK L2 + Gram per valid image row; st evicts on
            # Scalar; Gram lags a row; one conv1 half-thunk per row on DVE
            for e in range(e0, e1):
                off = (e - e0) * 128
                st = stk.tile([128, HEADS, 4, DH], BF16, tag="st")
                for half in range(2):
                    ps = psQ.tile([128, 2, 256], F32, tag="psQ")
                    for s2 in range(2):
                        hh, w2T = ((khy, "kw2T"), (qhx, "qw2T"),
                                   (khx, "kw2T"), (qhy, "qw2T"))[half * 2 + s2]
                        for k in range(2):
                            nc.tensor.matmul(ps[:, s2, :],
                                             hh[:, k, off:off + 128],
                                             w[w2T][:, k, :], start=(k == 0),
                                             stop=(k == 1))
                    nc.scalar.copy(
                        st[:, :, half * 2:half * 2 + 2, :],
                        ps.rearrange("p s (h d) -> p h s d", h=HEADS))
                if prev_st is not None:
                    emit_gram(*prev_st)
                prev_st = (st, vrow)
                vrow += 1
                if t < NT - 1:
                    drain_one()

            for g0, g1, after in C1CHUNKS:
                if after == t:
                    push_chunk(gx, vsp_x, g0, g1)
                    push_chunk(gy, vsp_y, g0, g1)

        emit_gram(*prev_st)

        # ================= Gram -> AllReduce =================
        # compact payload: per head, slot 0 = the four diagonal norm
        # blocks (stream s at partitions 32s), slot 1 = the off-diagonal
        # logits block (ky^Tqx for x heads at partitions 0:32, kx^Tqy for
        # y heads at 64:96). 64KB instead of the full 512KB Gram.
        gsb = sm.tile([128, 8, 2, 32], F32, tag="gsb")
        nc.scalar.memzero(gsb[:])
        for jt, gr in enumerate(grams):
            grv = gr.rearrange("p (m e) -> p m e", e=128)
            for s4 in range(4):
                nc.scalar.copy(
                    gsb[32 * s4:32 * s4 + 32, 4 * jt:4 * jt + 4, 0, :],
                    grv[32 * s4:32 * s4 + 32, :, 32 * s4:32 * s4 + 32])
            nc.scalar.copy(gsb[0:32, 4 * jt:4 * jt + 4, 1, :],
                           grv[0:32, :, 32:64])
            nc.scalar.copy(gsb[64:96, 4 * jt:4 * jt + 4, 1, :],
                           grv[64:96, :, 96:128])
        nc.sync.dma_start(cc_in.ap(), gsb[:])
        nc.gpsimd.collective_compute(
            "AllReduce", OP.add,
            ins=[cc_in.ap()], outs=[cc_out.ap()],
            replica_groups=[[0, 1, 2, 3], [4, 5, 6, 7]])

        # last conv1 chunk + boundary masking overlap the collective
        for g0, g1, after in C1CHUNKS:
            if after is None:
                push_chunk(gx, vsp_x, g0, g1)
                push_chunk(gy, vsp_y, g0, g1)
        while c1q:
            drain_one()
        for gbuf in (gx, gy):
            nc.vector.tensor_scalar_mul(gbuf[:, :, 0, :], gbuf[:, :, 0, :],
                                        w["gm0"][:])
            nc.vector.tensor_scalar_mul(gbuf[:, :, ER - 3, :],
                                        gbuf[:, :, ER - 3, :], w["gm33"][:])

        # softmax input DMAs (issued early; they wait on the collective)
        # dg index: 0=(x,g0) 1=(x,g1) 2=(y,g0) 3=(y,g1)
        # (P0 = off-diag partition base, K0/Q0 = k/q diag partition bases)
        s_t = sm.tile([128, 4, DH], F32, tag="s_t")
        db = sm.tile([128, 4, 2, DH], F32, tag="db")
        for dg in range(4):
            g = dg % 2
            P0 = 0 if dg < 2 else 64
            K0, Q0 = P0, P0 + 32
            def _blk(p0, slot):
                return cc_out.ap()[p0:p0 + 32, 4 * g:4 * g + 4, slot, :] \
                    .rearrange("d j e -> j d e")
            nc.sync.dma_start(s_t[:, dg, :], _blk(P0, 1))
            nc.sync.dma_start(db[:, dg, 0, :], _blk(K0, 0))
            nc.sync.dma_start(db[:, dg, 1, :], _blk(Q0, 0))

        # ========== final phase: conv2 groups (PE, evicted to SBUF) =======
        pairs = [(d, tt) for tt in range(8) for d in ("x", "y")]
        FIN = {"x": (vsp_x, gx, "obx", out_x), "y": (vsp_y, gy, "oby", out_y)}
        cparts = {}

        def emit_conv2(i):
            d, tt = pairs[i]
            vsp, gbuf, ob, o_dram = FIN[d]
            cpt = cp.tile([128, 2, 512], F16, tag="cpart", name="cpart")
            for mo in range(2):
                ps = psA.tile([128, 512], F32, tag="psA", name="finA")
                for i9, (dr, dc) in enumerate(TAPS):
                    src = gbuf[:, mo, 4 * tt + 1 + dr:4 * tt + 5 + dr,
                               1 + dc:129 + dc]
                    nc.tensor.matmul(ps[:], w["dw2"][:, mo, i9, :],
                                     src, start=(i9 == 0), stop=(i9 == 8),
                                     skip_group_check=True)
                nc.scalar.activation(cpt[:, mo, :], ps[:], AF.Identity,
                                     bias=w[ob][:, mo:mo + 1])
            cparts[i] = cpt

        vts = {}

        def issue_vt(i):
            d, tt = pairs[i]
            vt = vtp.tile([128, 2, 4 * WP], F16, tag="vt_f", name="vt_f")
            nc.sync.dma_start(
                vt[:],
                FIN[d][0].ap()[:, :, (4 * tt + 2) * WP:(4 * tt + 6) * WP])
            vts[i] = vt

        projb_i = [0]

        def proj_bank():
            j = projb_i[0] % 4
            projb_i[0] += 1
            if j < 2:
                return psQ.tile([128, 2, 256], F32, tag="psQ",
                                name="finQ").rearrange("p a b -> p (a b)")
            return grams[j - 2][:]

        def emit_proj(i):
            d, tt = pairs[i]
            o_dram = FIN[d][3]
            m1t = m1ts[d]
            vt = vts.pop(i)
            cpt = cparts.pop(i)
            o_t = ot.tile([128, 2, 4, 128], F16, tag="o_t")
            for mo in range(2):
                ps = proj_bank()
                for ke in range(2):
                    rhs = vt[:, ke, :].rearrange(
                        "p (r c) -> p r c", c=WP)[:, :, 1:129]
                    nc.tensor.matmul(ps, m1t[:, ke, mo, :], rhs,
                                     start=(ke == 0), stop=False,
                                     skip_group_check=True)
                nc.tensor.matmul(ps, w["eye128"][:], cpt[:, mo, :],
                                 start=False, stop=True,
                                 skip_group_check=True)
                nc.scalar.copy(o_t[:, mo, :, :],
                               ps.rearrange("p (r c) -> p r c", c=128))
            nc.sync.dma_start(
                o_dram.ap()[:, :, tt * 512:(tt + 1) * 512],
                o_t.rearrange("p a r c -> p a (r c)"))

        # ten conv2 groups precede the softmax on the PE queue: they run
        # through the AllReduce window (psA rotation, 2 groups in flight)
        emitted = 0
        while emitted < 12:
            emit_conv2(emitted)
            emitted += 1

        # ====== softmax + BD + fused proj matrices (x and y batched) ======
        dbv = db.rearrange("p a b d -> p (a b) d")
        nc.vector.tensor_tensor(dbv[:], dbv[:], w["eye8"][:], OP.mult)
        nkq = sm.tile([128, 4, 2], F32, tag="nkq")
        nc.vector.tensor_reduce(nkq.rearrange("p a b -> p (a b)")[:],
                                dbv[:], mybir.AxisListType.X, OP.add)
        inv = sm.tile([128, 4, 2], F32, tag="inv")
        nc.scalar.sqrt(inv[:], nkq[:])
        nc.vector.reciprocal(inv[:], inv[:])
        ks = sm.tile([128, 4], F32, tag="ks")
        nc.vector.tensor_tensor(ks[:], inv[:, :, 0], w["rxy_exp"][:], OP.mult)
        # qs[p, dg, j] = 1/||q_(head group(p), j)|| broadcast via blk128
        ei = sm.tile([128, 4, DH], F32R, tag="ei")
        for dg in range(4):
            nc.vector.tensor_scalar_mul(ei[:, dg, :], w["eye32r"][:],
                                        inv[:, dg, 1:2])
        pq = psQ.tile([128, 4, DH], F32, tag="psQ")
        nc.tensor.matmul(pq.rearrange("p a d -> p (a d)")[:], w["blk128"][:],
                         ei.rearrange("p a d -> p (a d)")[:],
                         start=True, stop=True)
        qks = sm.tile([128, 4, DH], F32, tag="qks")
        for dg in range(4):
            if dg % 2 == 0:
                nc.scalar.activation(qks[:, dg, :], pq[:, dg, :],
                                     AF.Identity, scale=ks[:, dg:dg + 1])
            else:
                nc.vector.tensor_scalar_mul(qks[:, dg, :], pq[:, dg, :],
                                            ks[:, dg:dg + 1])
        lg = sm.tile([128, 4, DH], F32, tag="lg")
        nc.vector.tensor_tensor(lg[:], s_t[:], qks[:], OP.mult)
        mx = sm.tile([128, 4], F32, tag="mx")
        nc.vector.tensor_reduce(mx[:], lg[:], mybir.AxisListType.X, OP.max)
        nc.vector.tensor_scalar_mul(mx[:], mx[:], -1.0)
        pe_ = sm.tile([128, 4, DH], F32, tag="pe_")
        ssum = sm.tile([128, 4], F32, tag="ssum")
        for dg in range(4):
            nc.scalar.activation(pe_[:, dg, :], lg[:, dg, :], AF.Exp,
                                 bias=mx[:, dg:dg + 1],
                                 accum_out=ssum[:, dg:dg + 1])
        nc.vector.reciprocal(ssum[:], ssum[:])
        at = sm.tile([128, 4, DH], F32, tag="at")
        for dg in range(4):
            if dg % 2 == 0:
                nc.vector.tensor_scalar_mul(at[:, dg, :], pe_[:, dg, :],
                                            ssum[:, dg:dg + 1])
            else:
                nc.scalar.activation(at[:, dg, :], pe_[:, dg, :],
                                     AF.Identity,
                                     scale=ssum[:, dg:dg + 1])
        m1ts = {}
        for d, (dgb, pwT) in {"x": (0, "pxwT"), "y": (2, "pywT")}.items():
            bds = sm.tile([128, 2, 256], F32, tag="bds")
            nc.vector.memset(bds[:], 0.0)
            for g in range(2):
                for j in range(4):
                    h = 4 * g + j
                    dst = bds[j * DH:(j + 1) * DH, g, h * DH:(h + 1) * DH]
                    src = at[j * DH:(j + 1) * DH, dgb + g, :]
                    if j % 2 == 0:
                        nc.vector.tensor_copy(dst, src)
                    else:
                        nc.scalar.copy(dst, src)
            bd = sm.tile([128, 2, 256], F32R, tag="bd")
            nc.vector.tensor_copy(bd[:], bds[:])
            m1t = sm.tile([128, 2, 2, 128], F16, tag=f"m1t_{d}")
            for me in range(2):
                ps = psQ.tile([128, 256], F32, tag="psQ")
                for g in range(2):
                    nc.tensor.matmul(ps[:],
                                     bd[:, g, me * 128:me * 128 + 128],
                                     w[pwT][:, g, :], start=(g == 0),
                                     stop=(g == 1))
                nc.scalar.copy(m1t[:, me, :, :],
                               ps.rearrange("p (a b) -> p a b", a=2))
            m1ts[d] = m1t

        # projection pass: proj pair i (+DVE add of its conv2 part), then
        # conv2 of pair i+10
        for i in range(3):
            issue_vt(i)
        for i in range(len(pairs)):
            while emitted < min(i + 13, len(pairs)):
                emit_conv2(emitted)
                emitted += 1
            if i + 3 < len(pairs):
                issue_vt(i + 3)
            emit_proj(i)

    nc.finalize()
    return nc


# ======================= host side =======================

def _prep_core_input(full, b, h0):
    """(H, W, C) rows [h0-2, h0+34) -> channel-major [128, 2, EN] f32
    (zeros outside the image)."""
    arr = np.zeros((ER, W, C), np.float32)
    r0, r1 = h0 - 2, h0 + RB + 2
    cr0, cr1 = max(r0, 0), min(r1, H)
    arr[cr0 - r0:cr1 - r0] = full[b, cr0:cr1]
    cm = arr.transpose(2, 0, 1).reshape(2, 128, EN)
    return np.ascontiguousarray(cm.transpose(1, 0, 2))


def _cm(v):
    return np.ascontiguousarray(v.reshape(2, 128).T.astype(np.float32))


def _lhsT(wm, nk):
    t = wm.T.reshape(nk, 128, 2, 128)
    return np.ascontiguousarray(t.transpose(1, 0, 2, 3).astype(np.float32))


def _rhsT(wm, dt=np.float32):
    t = wm.T.reshape(2, 128, wm.shape[0])
    return np.ascontiguousarray(t.transpose(1, 0, 2).astype(dt))


def kernel(_trace=False, **inputs):
    inp = {k: np.asarray(v) for k, v in inputs.items()}
    bf = ml_dtypes.bfloat16

    w2c = inp["pe_w2"].reshape(256, 9).astype(np.float32)
    dw2 = np.zeros((128, 2, 9, 128), np.float32)
    for g in range(2):
        for t in range(9):
            dw2[np.arange(128), g, t, np.arange(128)] = \
                w2c[g * 128:(g + 1) * 128, t]

    wvals = {
        "fxw1T": _lhsT(inp["fx_w1"], 4), "fyw1T": _lhsT(inp["fy_w1"], 4),
        "qw1T": _lhsT(inp["q_w1"], 2), "vw1T": _lhsT(inp["v_w1"], 2),
        "kxw1T": _lhsT(inp["k_w1"] @ inp["fx_w2"], 2),
        "kyw1T": _lhsT(inp["k_w1"] @ inp["fy_w2"], 2),
        "vw2T": _lhsT(inp["v_w2"], 2),
        "qw2T": _rhsT(inp["q_w2"], bf), "kw2T": _rhsT(inp["k_w2"], bf),
        "pxwT": _rhsT(inp["px_w"]), "pywT": _rhsT(inp["py_w"]),
        "dw2": dw2.astype(np.float16),
        "eye128": np.eye(128, dtype=np.float16),
        "blk128": np.kron(np.eye(4), np.ones((32, 32))).astype(np.float32),
        "eye32r": np.tile(np.eye(32), (4, 1)).astype(np.float32),
        "eye8": np.ascontiguousarray(
            np.broadcast_to(np.tile(np.eye(32), (4, 1))[:, None, :],
                            (128, 8, 32)).astype(np.float32)),
        "bfx": _cm(inp["fx_b1"]), "bfy": _cm(inp["fy_b1"]),
        "bq": _cm(inp["q_b1"]), "bv": _cm(inp["v_b1"]),
        "bkx": _cm(inp["k_w1"] @ inp["fx_b2"] + inp["k_b1"]),
        "bky": _cm(inp["k_w1"] @ inp["fy_b2"] + inp["k_b1"]),
        "obx": _cm(inp["px_b"] + inp["pe_b2"]),
        "oby": _cm(inp["py_b"] + inp["pe_b2"]),
        "w1c": np.ascontiguousarray(
            inp["pe_w1"].reshape(256, 9).reshape(2, 128, 9)
            .transpose(1, 0, 2).astype(np.float32)),
        "b1c": _cm(inp["pe_b1"]),
        "rxy_exp": np.ascontiguousarray(np.concatenate([
            np.repeat(inp["rescale_x"].reshape(2, 4), 32, axis=1).T,
            np.repeat(inp["rescale_y"].reshape(2, 4), 32, axis=1).T,
        ], axis=1).astype(np.float32)),
    }
    npdt = {"s": np.float32, "r": np.float32, "f": np.float32, "b": bf,
            "h": np.float16}
    blobs = {}
    for g, dt in npdt.items():
        parts = [wvals[n].reshape(128, -1).astype(dt)
                 for n, (gg, _) in WSPEC.items() if gg == g]
        blobs[g] = np.ascontiguousarray(np.concatenate(parts, axis=1))

    in_maps = []
    for r in range(8):
        b, h0 = r // 4, (r % 4) * RB
        m = {f"wg_{g}": blobs[g] for g in npdt}
        m["xin"] = _prep_core_input(inp["x_in"], b, h0)
        m["yin"] = _prep_core_input(inp["y_in"], b, h0)
        m["gm0"] = np.full((128, 1), 0.0 if h0 == 0 else 1.0, np.float32)
        m["gm33"] = np.full((128, 1), 0.0 if h0 + RB == H else 1.0,
                            np.float32)
        in_maps.append(m)

    if "nc" not in _CACHED:
        _CACHED["nc"] = _nc_build()
    res = run_bass_kernel_spmd(_CACHED["nc"], in_maps,
                               core_ids=list(range(8)), trace=_trace)
    _CACHED["last_result"] = res

    out_x = np.empty((B, H, W, C), np.float32)
    out_y = np.empty((B, H, W, C), np.float32)
    for r in range(8):
        b, h0 = r // 4, (r % 4) * RB
        for name, dst in (("out_x", out_x), ("out_y", out_y)):
            a = res.results[r][name].astype(np.float32)
            a = a.reshape(128, 2, RB, W)
            dst[b, h0:h0 + RB] = a.transpose(2, 3, 1, 0).reshape(RB, W, C)
    return out_x, out_y


# revision 33
# speedup vs baseline: 1.0056x; 1.0056x over previous
"""DMSA (dual-modal channel cross-attention) Trainium2 kernel — v5.

Sharding: 8 cores = 2 batches x 4 bands of 32 image rows. Each core
computes its band fully; the channel attention's per-head Gram matrices
(contraction over all n = h*w tokens, with l2-normalization folded in
via the Gram diagonal) are summed with one AllReduce per 4-core group.

Device layout: channel-major activations [128 partitions, 2 channel
halves, tokens]. Stage-1 runs on an unpadded 36x128 ext-row grid
(9 tiles x 512 tokens); v is spilled in fp16 to a width-padded 36x130
DRAM grid (pad columns kept zero inside the spill tile) so both 3x3
depthwise convs read taps as plain offset views.

Engines: big matmuls f32r; q/k path bf16; v/conv path fp16. conv1 runs
on DVE in half-chunk thunks drained one per image row so PSUM evicts
never queue behind it. conv2 runs as 9 diagonal PE matmuls per output
group, evicted to SBUF (bias folded) so ten groups precede the softmax
on the PE queue and cover the AllReduce window; the projection pass
then adds on top via DVE. Weights arrive in 5 blob DMAs.
"""
import numpy as np
import ml_dtypes
from contextlib import ExitStack

import concourse.bass as bass
import concourse.tile as tile
import concourse.mybir as mybir
from concourse import bacc
from concourse.bass_utils import run_bass_kernel_spmd

F32 = mybir.dt.float32
F32R = mybir.dt.float32r
BF16 = mybir.dt.bfloat16
F16 = mybir.dt.float16
AF = mybir.ActivationFunctionType
OP = mybir.AluOpType

B, H, W, C = 2, 128, 128, 256
HEADS, DH = 8, 32
RB = 32             # image rows per core
ER = RB + 4         # ext rows
WP = W + 2          # padded width (conv grid)
GN = ER * WP        # padded tokens (v spill grid) = 4680
EN = ER * W         # unpadded ext tokens (stage-1 grid) = 4608
NV = RB * W         # valid tokens = 4096
NT = 9              # stage-1 tiles (4 ext rows each)
LRELU_A = 0.01
# conv1 chunk g-row ranges and the stage-1 tile after which each may run
C1CHUNKS = [(0, 6, 1), (6, 12, 3), (12, 18, 4), (18, 24, 6), (24, 30, 7),
            (30, 34, None)]  # None -> after the collective

# weight blobs: name -> (group, shape); group s = hot f32r (tile-0 deps)
WSPEC = {
    "fxw1T": ("s", [128, 4, 2, 128]), "fyw1T": ("s", [128, 4, 2, 128]),
    "qw1T": ("s", [128, 2, 2, 128]), "vw1T": ("s", [128, 2, 2, 128]),
    "vw2T": ("r", [128, 2, 2, 128]), "pxwT": ("r", [128, 2, 256]),
    "pywT": ("r", [128, 2, 256]), "blk128": ("r", [128, 128]),
    "eye32r": ("f", [128, 32]), "eye8": ("f", [128, 8, 32]),
    "bfx": ("f", [128, 2]), "bfy": ("f", [128, 2]), "bq": ("f", [128, 2]),
    "bkx": ("f", [128, 2]), "bky": ("f", [128, 2]), "bv": ("f", [128, 2]),
    "obx": ("f", [128, 2]), "oby": ("f", [128, 2]), "b1c": ("f", [128, 2]),
    "rxy_exp": ("f", [128, 4]), "w1c": ("f", [128, 2, 9]),
    "qw2T": ("b", [128, 2, 256]), "kw2T": ("b", [128, 2, 256]),
    "kxw1T": ("b", [128, 2, 2, 128]), "kyw1T": ("b", [128, 2, 2, 128]),
    "dw2": ("h", [128, 2, 9, 128]), "eye128": ("h", [128, 128]),
}
GDT = {"s": F32R, "r": F32R, "f": F32, "b": BF16, "h": F16}

_CACHED = {}


def _nc_build():
    nc = bacc.Bacc(num_devices=8)

    gsz = {g: 0 for g in GDT}
    woff = {}
    for name, (g, shp) in WSPEC.items():
        n = int(np.prod(shp[1:]))
        woff[name] = gsz[g]
        gsz[g] += n

    din = {}
    for g in GDT:
        din[g] = nc.dram_tensor(f"wg_{g}", [128, gsz[g]], GDT[g],
                                kind="ExternalInput")
    xin = nc.dram_tensor("xin", [128, 2, EN], F32R, kind="ExternalInput")
    yin = nc.dram_tensor("yin", [128, 2, EN], F32R, kind="ExternalInput")
    gm0_t = nc.dram_tensor("gm0", [128, 1], F32, kind="ExternalInput")
    gm33_t = nc.dram_tensor("gm33", [128, 1], F32, kind="ExternalInput")

    out_x = nc.dram_tensor("out_x", [128, 2, NV], F16, kind="ExternalOutput")
    out_y = nc.dram_tensor("out_y", [128, 2, NV], F16, kind="ExternalOutput")
    cc_in = nc.dram_tensor("cc_in", [128, HEADS, 2, 32], F32,
                           kind="Internal")
    cc_out = nc.dram_tensor("cc_out", [128, HEADS, 2, 32], F32,
                            kind="Internal")

    with tile.TileContext(nc) as tc, ExitStack() as ctx:
        wp = ctx.enter_context(tc.tile_pool(name="wp", bufs=1))
        io = ctx.enter_context(tc.tile_pool(name="io", bufs=2))
        hidF = ctx.enter_context(tc.tile_pool(name="hidF", bufs=2))
        hidQ = ctx.enter_context(tc.tile_pool(name="hidQ", bufs=2))
        hidV = ctx.enter_context(tc.tile_pool(name="hidV", bufs=2))
        stk = ctx.enter_context(tc.tile_pool(name="stk", bufs=2))
        sm = ctx.enter_context(tc.tile_pool(name="sm", bufs=1))
        gb = ctx.enter_context(tc.tile_pool(name="gb", bufs=1))
        cvp = ctx.enter_context(tc.tile_pool(name="cvp", bufs=2))
        ot = ctx.enter_context(tc.tile_pool(name="ot", bufs=2))
        cp = ctx.enter_context(tc.tile_pool(name="cp", bufs=12))
        psA = ctx.enter_context(tc.tile_pool(name="psA", bufs=4, space="PSUM"))
        psQ = ctx.enter_context(tc.tile_pool(name="psQ", bufs=2, space="PSUM"))
        psG = ctx.enter_context(tc.tile_pool(name="psG", bufs=1, space="PSUM"))

        # hot weights first, then the first input tiles, then other blobs
        w = {}
        wt = {}
        wt["s"] = wp.tile([128, gsz["s"]], GDT["s"], tag="wg_s",
                          name="wg_s")
        nc.sync.dma_start(wt["s"][:], din["s"].ap())
        xt0 = io.tile([128, 2, 512], F32R, tag="xt")
        nc.sync.dma_start(xt0[:], xin.ap()[:, :, 0:512])
        yt0 = io.tile([128, 2, 512], F32R, tag="yt")
        nc.sync.dma_start(yt0[:], yin.ap()[:, :, 0:512])
        for g in GDT:
            if g == "s":
                continue
            wt[g] = wp.tile([128, gsz[g]], GDT[g], tag=f"wg_{g}",
                            name=f"wg_{g}")
            nc.sync.dma_start(wt[g][:], din[g].ap())
        for name, (g, shp) in WSPEC.items():
            v = wt[g][:, woff[name]:woff[name] + int(np.prod(shp[1:]))]
            if len(shp) == 3:
                v = v.rearrange("p (a b) -> p a b", a=shp[1])
            elif len(shp) == 4:
                v = v.rearrange("p (a b c) -> p a b c", a=shp[1], b=shp[2])
            w[name] = v
        for name, t in (("gm0", gm0_t), ("gm33", gm33_t)):
            tl = wp.tile([128, 1], F32, tag=f"w_{name}", name=f"w_{name}")
            nc.sync.dma_start(tl[:], t.ap())
            w[name] = tl

        # warm the activation tables (independent ops so the loads
        # pipeline back-to-back; stage-1 functions first)
        warm = sm.tile([128, 12], F32, tag="warm")
        nc.vector.memset(warm[:], 1.0)
        wf16 = sm.tile([128, 4], F16, tag="wf16")
        wsum = sm.tile([128, 1], F32, tag="wsum")
        nc.scalar.activation(warm[:, 2:3], warm[:, 0:1], AF.Lrelu,
                             bias=warm[:, 1:2], alpha=LRELU_A)
        nc.scalar.activation(wf16[:, 0:1], warm[:, 0:1], AF.Gelu,
                             bias=warm[:, 1:2])
        nc.scalar.activation(wf16[:, 1:2], warm[:, 0:1], AF.Identity,
                             bias=warm[:, 1:2])
        nc.scalar.copy(warm[:, 3:4], warm[:, 0:1])
        nc.scalar.copy(wf16[:, 2:3], warm[:, 0:1])
        nc.scalar.sqrt(warm[:, 4:5], warm[:, 0:1])
        nc.scalar.activation(warm[:, 5:6], warm[:, 0:1], AF.Exp,
                             bias=warm[:, 1:2], accum_out=wsum[:])
        nc.scalar.activation(warm[:, 6:7], warm[:, 0:1], AF.Identity,
                             scale=warm[:, 1:2])
        nc.scalar.activation(warm[:, 7:8], warm[:, 0:1], AF.Gelu,
                             bias=warm[:, 1:2])

        gram0 = psG.tile([128, 512], F32, tag="gram0")
        gram1 = psG.tile([128, 512], F32, tag="gram1")
        grams = [gram0, gram1]

        gx = gb.tile([128, 2, ER - 2, WP], F16, tag="gx")
        gy = gb.tile([128, 2, ER - 2, WP], F16, tag="gy")
        nc.scalar.memzero(gx[:])
        nc.scalar.memzero(gy[:])
        TAPS = [(dr, dc) for dr in (-1, 0, 1) for dc in (-1, 0, 1)]

        # v lives in SBUF for its whole life (no DRAM spill): pad columns
        # zeroed once, interiors overwritten tile by tile
        vb_x = gb.tile([128, 2, ER, WP], F16, tag="vb_x")
        vb_y = gb.tile([128, 2, ER, WP], F16, tag="vb_y")
        for vb in (vb_x, vb_y):
            nc.vector.memset(vb[:, :, :, 0:1], 0.0)
            nc.vector.memset(vb[:, :, :, WP - 1:WP], 0.0)

        def conv1_half(vb, gbuf, g0, g1, g):
            """DVE 9-tap fp16 conv1 of one channel half + Gelu evict."""
            nr = g1 - g0
            acc = cvp.tile([128, 6, 128], F16, tag="cacc", name="cacc")
            for i, (dr, dc) in enumerate(TAPS):
                src = vb[:, g, g0 + 1 + dr:g0 + 1 + dr + nr,
                         1 + dc:129 + dc]
                if i == 0:
                    nc.vector.tensor_scalar_mul(acc[:, :nr, :], src,
                                                w["w1c"][:, g, 0:1])
                else:
                    nc.vector.scalar_tensor_tensor(
                        acc[:, :nr, :], src, w["w1c"][:, g, i:i + 1],
                        acc[:, :nr, :], OP.mult, OP.add)
            nc.scalar.activation(gbuf[:, g, g0:g1, 1:129], acc[:, :nr, :],
                                 AF.Gelu, bias=w["b1c"][:, g:g + 1])

        c1q = []

        def push_chunk(gbuf, vb, g0, g1):
            c1q.append(lambda: conv1_half(vb, gbuf, g0, g1, 0))
            c1q.append(lambda: conv1_half(vb, gbuf, g0, g1, 1))

        def drain_one():
            if c1q:
                c1q.pop(0)()

        # ================= stage 1 =================
        vrow = 0
        prev_st = None

        def emit_gram(st, row):
            for h in range(HEADS):
                nc.tensor.matmul(
                    grams[h // 4][:, (h % 4) * 128:(h % 4) * 128 + 128],
                    st[:, h], st[:, h],
                    start=(row == 0), stop=(row == RB - 1),
                    skip_group_check=True)

        def mlp1(srcs, w1T, nk, bias, tag, pool, dt, lo=0, n=512):
            """hidden = lrelu(srcs @ w1T + b); per-half PSUM banks."""
            ht = pool.tile([128, 2, 512], dt, tag=tag)
            for mh in range(2):
                ps = psA.tile([128, 512], F32, tag="psA")
                for k in range(nk):
                    src = srcs[k // 2][:, k % 2, lo:lo + n] if len(srcs) > 1 \
                        else srcs[0][:, k, lo:lo + n]
                    nc.tensor.matmul(ps[:, :n], w1T[:, k, mh, :], src,
                                     start=(k == 0), stop=(k == nk - 1))
                nc.scalar.activation(ht[:, mh, :n], ps[:, :n], AF.Lrelu,
                                     bias=bias[:, mh:mh + 1], alpha=LRELU_A)
            return ht

        for t in range(NT):
            if t == 0:
                xt, yt = xt0, yt0
            else:
                xt = io.tile([128, 2, 512], F32R, tag="xt")
                nc.sync.dma_start(xt[:],
                                  xin.ap()[:, :, t * 512:(t + 1) * 512])
                yt = io.tile([128, 2, 512], F32R, tag="yt")
                nc.sync.dma_start(yt[:],
                                  yin.ap()[:, :, t * 512:(t + 1) * 512])

            # valid-row window within this tile
            e0, e1 = max(2, 4 * t), min(ER - 2, 4 * t + 4)
            lo, n = (e0 - 4 * t) * 128, (e1 - e0) * 128

            fhx = mlp1([xt, yt], w["fxw1T"], 4, w["bfx"], "fhx", hidF, BF16,
                       lo, n)
            fhy = mlp1([xt, yt], w["fyw1T"], 4, w["bfy"], "fhy", hidF, BF16,
                       lo, n)
            qhx = mlp1([xt], w["qw1T"], 2, w["bq"], "qhx", hidQ, BF16, lo, n)
            qhy = mlp1([yt], w["qw1T"], 2, w["bq"], "qhy", hidQ, BF16, lo, n)
            khx = mlp1([fhx], w["kxw1T"], 2, w["bkx"], "khx", hidQ, BF16,
                       0, n)
            khy = mlp1([fhy], w["kyw1T"], 2, w["bky"], "khy", hidQ, BF16,
                       0, n)
            vhx = mlp1([xt], w["vw1T"], 2, w["bv"], "vhx", hidV, F32R)
            vhy = mlp1([yt], w["vw1T"], 2, w["bv"], "vhy", hidV, F32R)

            # v = vhid @ vw2T (ext tokens), evicted straight into SBUF vb
            for nm, vh, vb in (("x", vhx, vb_x), ("y", vhy, vb_y)):
                for g in range(2):
                    ps = psA.tile([128, 512], F32, tag="psA")
                    for k in range(2):
                        nc.tensor.matmul(ps[:], w["vw2T"][:, k, g, :],
                                         vh[:, k, :], start=(k == 0),
                                         stop=(k == 1))
                    nc.vector.tensor_copy(
                        vb[:, g, 4 * t:4 * t + 4, 1:129],
                        ps.rearrange("p (r c) -> p r c", c=128))

            # token-major QK L2 + Gram per valid image row; st evicts on
            # Scalar; Gram lags a row; one conv1 half-thunk per row on DVE
            for e in range(e0, e1):
                off = (e - e0) * 128
                st = stk.tile([128, HEADS, 4, DH], BF16, tag="st")
                for half in range(2):
                    ps = psQ.tile([128, 2, 256], F32, tag="psQ")
                    for s2 in range(2):
                        hh, w2T = ((khy, "kw2T"), (qhx, "qw2T"),
                                   (khx, "kw2T"), (qhy, "qw2T"))[half * 2 + s2]
                        for k in range(2):
                            nc.tensor.matmul(ps[:, s2, :],
                                             hh[:, k, off:off + 128],
                                             w[w2T][:, k, :], start=(k == 0),
                                             stop=(k == 1))
                    nc.scalar.copy(
                        st[:, :, half * 2:half * 2 + 2, :],
                        ps.rearrange("p s (h d) -> p h s d", h=HEADS))
                if prev_st is not None:
                    emit_gram(*prev_st)
                prev_st = (st, vrow)
                vrow += 1
                if t < NT - 1:
                    drain_one()

            for g0, g1, after in C1CHUNKS:
                if after == t:
                    push_chunk(gx, vb_x, g0, g1)
                    push_chunk(gy, vb_y, g0, g1)

        emit_gram(*prev_st)

        # ================= Gram -> AllReduce =================
        # compact payload: per head, slot 0 = the four diagonal norm
        # blocks (stream s at partitions 32s), slot 1 = the off-diagonal
        # logits block (ky^Tqx for x heads at partitions 0:32, kx^Tqy for
        # y heads at 64:96). 64KB instead of the full 512KB Gram.
        gsb = sm.tile([128, 8, 2, 32], F32, tag="gsb")
        nc.scalar.memzero(gsb[:])
        for jt, gr in enumerate(grams):
            grv = gr.rearrange("p (m e) -> p m e", e=128)
            for s4 in range(4):
                nc.scalar.copy(
                    gsb[32 * s4:32 * s4 + 32, 4 * jt:4 * jt + 4, 0, :],
                    grv[32 * s4:32 * s4 + 32, :, 32 * s4:32 * s4 + 32])
            nc.scalar.copy(gsb[0:32, 4 * jt:4 * jt + 4, 1, :],
                           grv[0:32, :, 32:64])
            nc.scalar.copy(gsb[64:96, 4 * jt:4 * jt + 4, 1, :],
                           grv[64:96, :, 96:128])
        nc.sync.dma_start(cc_in.ap(), gsb[:])
        nc.gpsimd.collective_compute(
            "AllReduce", OP.add,
            ins=[cc_in.ap()], outs=[cc_out.ap()],
            replica_groups=[[0, 1, 2, 3], [4, 5, 6, 7]])

        # last conv1 chunk + boundary masking overlap the collective
        for g0, g1, after in C1CHUNKS:
            if after is None:
                push_chunk(gx, vb_x, g0, g1)
                push_chunk(gy, vb_y, g0, g1)
        while c1q:
            drain_one()
        for gbuf in (gx, gy):
            nc.vector.tensor_scalar_mul(gbuf[:, :, 0, :], gbuf[:, :, 0, :],
                                        w["gm0"][:])
            nc.vector.tensor_scalar_mul(gbuf[:, :, ER - 3, :],
                                        gbuf[:, :, ER - 3, :], w["gm33"][:])

        # softmax input DMAs (issued early; they wait on the collective)
        # dg index: 0=(x,g0) 1=(x,g1) 2=(y,g0) 3=(y,g1)
        # (P0 = off-diag partition base, K0/Q0 = k/q diag partition bases)
        s_t = sm.tile([128, 4, DH], F32, tag="s_t")
        db = sm.tile([128, 4, 2, DH], F32, tag="db")
        for dg in range(4):
            g = dg % 2
            P0 = 0 if dg < 2 else 64
            K0, Q0 = P0, P0 + 32
            def _blk(p0, slot):
                return cc_out.ap()[p0:p0 + 32, 4 * g:4 * g + 4, slot, :] \
                    .rearrange("d j e -> j d e")
            nc.sync.dma_start(s_t[:, dg, :], _blk(P0, 1))
            nc.sync.dma_start(db[:, dg, 0, :], _blk(K0, 0))
            nc.sync.dma_start(db[:, dg, 1, :], _blk(Q0, 0))

        # ========== final phase: conv2 groups (PE, evicted to SBUF) =======
        pairs = [(d, tt) for tt in range(8) for d in ("x", "y")]
        FIN = {"x": (vb_x, gx, "obx", out_x), "y": (vb_y, gy, "oby", out_y)}
        cparts = {}

        def emit_conv2(i):
            d, tt = pairs[i]
            vsp, gbuf, ob, o_dram = FIN[d]
            cpt = cp.tile([128, 2, 512], F16, tag="cpart", name="cpart")
            for mo in range(2):
                ps = psA.tile([128, 512], F32, tag="psA", name="finA")
                for i9, (dr, dc) in enumerate(TAPS):
                    src = gbuf[:, mo, 4 * tt + 1 + dr:4 * tt + 5 + dr,
                               1 + dc:129 + dc]
                    nc.tensor.matmul(ps[:], w["dw2"][:, mo, i9, :],
                                     src, start=(i9 == 0), stop=(i9 == 8),
                                     skip_group_check=True)
                nc.scalar.activation(cpt[:, mo, :], ps[:], AF.Identity,
                                     bias=w[ob][:, mo:mo + 1])
            cparts[i] = cpt

        projb_i = [0]

        def proj_bank():
            j = projb_i[0] % 4
            projb_i[0] += 1
            if j < 2:
                return psQ.tile([128, 2, 256], F32, tag="psQ",
                                name="finQ").rearrange("p a b -> p (a b)")
            return grams[j - 2][:]

        def emit_proj(i):
            d, tt = pairs[i]
            vb, _, _, o_dram = FIN[d]
            m1t = m1ts[d]
            cpt = cparts.pop(i)
            o_t = ot.tile([128, 2, 4, 128], F16, tag="o_t")
            for mo in range(2):
                ps = proj_bank()
                for ke in range(2):
                    rhs = vb[:, ke, 4 * tt + 2:4 * tt + 6, 1:129]
                    nc.tensor.matmul(ps, m1t[:, ke, mo, :], rhs,
                                     start=(ke == 0), stop=False,
                                     skip_group_check=True)
                nc.tensor.matmul(ps, w["eye128"][:], cpt[:, mo, :],
                                 start=False, stop=True,
                                 skip_group_check=True)
                nc.scalar.copy(o_t[:, mo, :, :],
                               ps.rearrange("p (r c) -> p r c", c=128))
            nc.sync.dma_start(
                o_dram.ap()[:, :, tt * 512:(tt + 1) * 512],
                o_t.rearrange("p a r c -> p a (r c)"))

        # ten conv2 groups precede the softmax on the PE queue: they run
        # through the AllReduce window (psA rotation, 2 groups in flight)
        emitted = 0
        while emitted < 12:
            emit_conv2(emitted)
            emitted += 1

        # ====== softmax + BD + fused proj matrices (x and y batched) ======
        dbv = db.rearrange("p a b d -> p (a b) d")
        nc.vector.tensor_tensor(dbv[:], dbv[:], w["eye8"][:], OP.mult)
        nkq = sm.tile([128, 4, 2], F32, tag="nkq")
        nc.vector.tensor_reduce(nkq.rearrange("p a b -> p (a b)")[:],
                                dbv[:], mybir.AxisListType.X, OP.add)
        inv = sm.tile([128, 4, 2], F32, tag="inv")
        nc.scalar.sqrt(inv[:], nkq[:])
        nc.vector.reciprocal(inv[:], inv[:])
        ks = sm.tile([128, 4], F32, tag="ks")
        nc.vector.tensor_tensor(ks[:], inv[:, :, 0], w["rxy_exp"][:], OP.mult)
        # qs[p, dg, j] = 1/||q_(head group(p), j)|| broadcast via blk128
        ei = sm.tile([128, 4, DH], F32R, tag="ei")
        for dg in range(4):
            nc.vector.tensor_scalar_mul(ei[:, dg, :], w["eye32r"][:],
                                        inv[:, dg, 1:2])
        pq = psQ.tile([128, 4, DH], F32, tag="psQ")
        nc.tensor.matmul(pq.rearrange("p a d -> p (a d)")[:], w["blk128"][:],
                         ei.rearrange("p a d -> p (a d)")[:],
                         start=True, stop=True)
        qks = sm.tile([128, 4, DH], F32, tag="qks")
        for dg in range(4):
            if dg % 2 == 0:
                nc.scalar.activation(qks[:, dg, :], pq[:, dg, :],
                                     AF.Identity, scale=ks[:, dg:dg + 1])
            else:
                nc.vector.tensor_scalar_mul(qks[:, dg, :], pq[:, dg, :],
                                            ks[:, dg:dg + 1])
        lg = sm.tile([128, 4, DH], F32, tag="lg")
        nc.vector.tensor_tensor(lg[:], s_t[:], qks[:], OP.mult)
        mx = sm.tile([128, 4], F32, tag="mx")
        nc.vector.tensor_reduce(mx[:], lg[:], mybir.AxisListType.X, OP.max)
        nc.vector.tensor_scalar_mul(mx[:], mx[:], -1.0)
        pe_ = sm.tile([128, 4, DH], F32, tag="pe_")
        ssum = sm.tile([128, 4], F32, tag="ssum")
        for dg in range(4):
            nc.scalar.activation(pe_[:, dg, :], lg[:, dg, :], AF.Exp,
                                 bias=mx[:, dg:dg + 1],
                                 accum_out=ssum[:, dg:dg + 1])
        nc.vector.reciprocal(ssum[:], ssum[:])
        at = sm.tile([128, 4, DH], F32, tag="at")
        for dg in range(4):
            if dg % 2 == 0:
                nc.vector.tensor_scalar_mul(at[:, dg, :], pe_[:, dg, :],
                                            ssum[:, dg:dg + 1])
            else:
                nc.scalar.activation(at[:, dg, :], pe_[:, dg, :],
                                     AF.Identity,
                                     scale=ssum[:, dg:dg + 1])
        m1ts = {}
        for d, (dgb, pwT) in {"x": (0, "pxwT"), "y": (2, "pywT")}.items():
            bds = sm.tile([128, 2, 256], F32, tag="bds")
            nc.vector.memset(bds[:], 0.0)
            for g in range(2):
                for j in range(4):
                    h = 4 * g + j
                    dst = bds[j * DH:(j + 1) * DH, g, h * DH:(h + 1) * DH]
                    src = at[j * DH:(j + 1) * DH, dgb + g, :]
                    if j % 2 == 0:
                        nc.vector.tensor_copy(dst, src)
                    else:
                        nc.scalar.copy(dst, src)
            bd = sm.tile([128, 2, 256], F32R, tag="bd")
            nc.vector.tensor_copy(bd[:], bds[:])
            m1t = sm.tile([128, 2, 2, 128], F16, tag=f"m1t_{d}")
            for me in range(2):
                ps = psQ.tile([128, 256], F32, tag="psQ")
                for g in range(2):
                    nc.tensor.matmul(ps[:],
                                     bd[:, g, me * 128:me * 128 + 128],
                                     w[pwT][:, g, :], start=(g == 0),
                                     stop=(g == 1))
                nc.scalar.copy(m1t[:, me, :, :],
                               ps.rearrange("p (a b) -> p a b", a=2))
            m1ts[d] = m1t

        # projection pass: proj pair i, then conv2 of pair i+12
        for i in range(len(pairs)):
            while emitted < min(i + 13, len(pairs)):
                emit_conv2(emitted)
                emitted += 1
            emit_proj(i)

    nc.finalize()
    return nc


# ======================= host side =======================

def _prep_core_input(full, b, h0):
    """(H, W, C) rows [h0-2, h0+34) -> channel-major [128, 2, EN] f32
    (zeros outside the image)."""
    arr = np.zeros((ER, W, C), np.float32)
    r0, r1 = h0 - 2, h0 + RB + 2
    cr0, cr1 = max(r0, 0), min(r1, H)
    arr[cr0 - r0:cr1 - r0] = full[b, cr0:cr1]
    cm = arr.transpose(2, 0, 1).reshape(2, 128, EN)
    return np.ascontiguousarray(cm.transpose(1, 0, 2))


def _cm(v):
    return np.ascontiguousarray(v.reshape(2, 128).T.astype(np.float32))


def _lhsT(wm, nk):
    t = wm.T.reshape(nk, 128, 2, 128)
    return np.ascontiguousarray(t.transpose(1, 0, 2, 3).astype(np.float32))


def _rhsT(wm, dt=np.float32):
    t = wm.T.reshape(2, 128, wm.shape[0])
    return np.ascontiguousarray(t.transpose(1, 0, 2).astype(dt))


def kernel(_trace=False, **inputs):
    inp = {k: np.asarray(v) for k, v in inputs.items()}
    bf = ml_dtypes.bfloat16

    w2c = inp["pe_w2"].reshape(256, 9).astype(np.float32)
    dw2 = np.zeros((128, 2, 9, 128), np.float32)
    for g in range(2):
        for t in range(9):
            dw2[np.arange(128), g, t, np.arange(128)] = \
                w2c[g * 128:(g + 1) * 128, t]

    wvals = {
        "fxw1T": _lhsT(inp["fx_w1"], 4), "fyw1T": _lhsT(inp["fy_w1"], 4),
        "qw1T": _lhsT(inp["q_w1"], 2), "vw1T": _lhsT(inp["v_w1"], 2),
        "kxw1T": _lhsT(inp["k_w1"] @ inp["fx_w2"], 2),
        "kyw1T": _lhsT(inp["k_w1"] @ inp["fy_w2"], 2),
        "vw2T": _lhsT(inp["v_w2"], 2),
        "qw2T": _rhsT(inp["q_w2"], bf), "kw2T": _rhsT(inp["k_w2"], bf),
        "pxwT": _rhsT(inp["px_w"]), "pywT": _rhsT(inp["py_w"]),
        "dw2": dw2.astype(np.float16),
        "eye128": np.eye(128, dtype=np.float16),
        "blk128": np.kron(np.eye(4), np.ones((32, 32))).astype(np.float32),
        "eye32r": np.tile(np.eye(32), (4, 1)).astype(np.float32),
        "eye8": np.ascontiguousarray(
            np.broadcast_to(np.tile(np.eye(32), (4, 1))[:, None, :],
                            (128, 8, 32)).astype(np.float32)),
        "bfx": _cm(inp["fx_b1"]), "bfy": _cm(inp["fy_b1"]),
        "bq": _cm(inp["q_b1"]), "bv": _cm(inp["v_b1"]),
        "bkx": _cm(inp["k_w1"] @ inp["fx_b2"] + inp["k_b1"]),
        "bky": _cm(inp["k_w1"] @ inp["fy_b2"] + inp["k_b1"]),
        "obx": _cm(inp["px_b"] + inp["pe_b2"]),
        "oby": _cm(inp["py_b"] + inp["pe_b2"]),
        "w1c": np.ascontiguousarray(
            inp["pe_w1"].reshape(256, 9).reshape(2, 128, 9)
            .transpose(1, 0, 2).astype(np.float32)),
        "b1c": _cm(inp["pe_b1"]),
        "rxy_exp": np.ascontiguousarray(np.concatenate([
            np.repeat(inp["rescale_x"].reshape(2, 4), 32, axis=1).T,
            np.repeat(inp["rescale_y"].reshape(2, 4), 32, axis=1).T,
        ], axis=1).astype(np.float32)),
    }
    npdt = {"s": np.float32, "r": np.float32, "f": np.float32, "b": bf,
            "h": np.float16}
    blobs = {}
    for g, dt in npdt.items():
        parts = [wvals[n].reshape(128, -1).astype(dt)
                 for n, (gg, _) in WSPEC.items() if gg == g]
        blobs[g] = np.ascontiguousarray(np.concatenate(parts, axis=1))

    in_maps = []
    for r in range(8):
        b, h0 = r // 4, (r % 4) * RB
        m = {f"wg_{g}": blobs[g] for g in npdt}
        m["xin"] = _prep_core_input(inp["x_in"], b, h0)
        m["yin"] = _prep_core_input(inp["y_in"], b, h0)
        m["gm0"] = np.full((128, 1), 0.0 if h0 == 0 else 1.0, np.float32)
        m["gm33"] = np.full((128, 1), 0.0 if h0 + RB == H else 1.0,
                            np.float32)
        in_maps.append(m)

    if "nc" not in _CACHED:
        _CACHED["nc"] = _nc_build()
    res = run_bass_kernel_spmd(_CACHED["nc"], in_maps,
                               core_ids=list(range(8)), trace=_trace)
    _CACHED["last_result"] = res

    out_x = np.empty((B, H, W, C), np.float32)
    out_y = np.empty((B, H, W, C), np.float32)
    for r in range(8):
        b, h0 = r // 4, (r % 4) * RB
        for name, dst in (("out_x", out_x), ("out_y", out_y)):
            a = res.results[r][name].astype(np.float32)
            a = a.reshape(128, 2, RB, W)
            dst[b, h0:h0 + RB] = a.transpose(2, 3, 1, 0).reshape(RB, W, C)
    return out_x, out_y


# revision 34
# speedup vs baseline: 1.0193x; 1.0137x over previous
"""DMSA (dual-modal channel cross-attention) Trainium2 kernel — v5.

Sharding: 8 cores = 2 batches x 4 bands of 32 image rows. Each core
computes its band fully; the channel attention's per-head Gram matrices
(contraction over all n = h*w tokens, with l2-normalization folded in
via the Gram diagonal) are summed with one AllReduce per 4-core group.

Device layout: channel-major activations [128 partitions, 2 channel
halves, tokens]. Stage-1 runs on an unpadded 36x128 ext-row grid
(9 tiles x 512 tokens); v is spilled in fp16 to a width-padded 36x130
DRAM grid (pad columns kept zero inside the spill tile) so both 3x3
depthwise convs read taps as plain offset views.

Engines: big matmuls f32r; q/k path bf16; v/conv path fp16. conv1 runs
on DVE in half-chunk thunks drained one per image row so PSUM evicts
never queue behind it. conv2 runs as 9 diagonal PE matmuls per output
group, evicted to SBUF (bias folded) so ten groups precede the softmax
on the PE queue and cover the AllReduce window; the projection pass
then adds on top via DVE. Weights arrive in 5 blob DMAs.
"""
import numpy as np
import ml_dtypes
from contextlib import ExitStack

import concourse.bass as bass
import concourse.tile as tile
import concourse.mybir as mybir
from concourse import bacc
from concourse.bass_utils import run_bass_kernel_spmd

F32 = mybir.dt.float32
F32R = mybir.dt.float32r
BF16 = mybir.dt.bfloat16
F16 = mybir.dt.float16
AF = mybir.ActivationFunctionType
OP = mybir.AluOpType

B, H, W, C = 2, 128, 128, 256
HEADS, DH = 8, 32
RB = 32             # image rows per core
ER = RB + 4         # ext rows
WP = W + 2          # padded width (conv grid)
GN = ER * WP        # padded tokens (v spill grid) = 4680
EN = ER * W         # unpadded ext tokens (stage-1 grid) = 4608
NV = RB * W         # valid tokens = 4096
NT = 9              # stage-1 tiles (4 ext rows each)
LRELU_A = 0.01
# conv1 chunk g-row ranges and the stage-1 tile after which each may run
C1CHUNKS = [(0, 6, 1), (6, 12, 3), (12, 18, 4), (18, 24, 6), (24, 30, 7),
            (30, 34, None)]  # None -> after the collective

# weight blobs: name -> (group, shape); group s = hot f32r (tile-0 deps)
WSPEC = {
    "fxw1T": ("s", [128, 4, 2, 128]), "fyw1T": ("s", [128, 4, 2, 128]),
    "qw1T": ("s", [128, 2, 2, 128]), "vw1T": ("s", [128, 2, 2, 128]),
    "vw2T": ("r", [128, 2, 2, 128]), "pxwT": ("r", [128, 2, 256]),
    "pywT": ("r", [128, 2, 256]), "blk128": ("r", [128, 128]),
    "eye32r": ("f", [128, 32]), "eye8": ("f", [128, 8, 32]),
    "bfx": ("f", [128, 2]), "bfy": ("f", [128, 2]), "bq": ("f", [128, 2]),
    "bkx": ("f", [128, 2]), "bky": ("f", [128, 2]), "bv": ("f", [128, 2]),
    "obx": ("f", [128, 2]), "oby": ("f", [128, 2]), "b1c": ("f", [128, 2]),
    "rxy_exp": ("f", [128, 4]), "w1c": ("f", [128, 2, 9]),
    "qw2T": ("b", [128, 2, 256]), "kw2T": ("b", [128, 2, 256]),
    "kxw1T": ("b", [128, 2, 2, 128]), "kyw1T": ("b", [128, 2, 2, 128]),
    "dw2": ("h", [128, 2, 9, 128]), "eye128": ("h", [128, 128]),
}
GDT = {"s": F32R, "r": F32R, "f": F32, "b": BF16, "h": F16}

_CACHED = {}


def _nc_build():
    nc = bacc.Bacc(num_devices=8)

    gsz = {g: 0 for g in GDT}
    woff = {}
    for name, (g, shp) in WSPEC.items():
        n = int(np.prod(shp[1:]))
        woff[name] = gsz[g]
        gsz[g] += n

    din = {}
    for g in GDT:
        din[g] = nc.dram_tensor(f"wg_{g}", [128, gsz[g]], GDT[g],
                                kind="ExternalInput")
    xin = nc.dram_tensor("xin", [128, 2, EN], F32R, kind="ExternalInput")
    yin = nc.dram_tensor("yin", [128, 2, EN], F32R, kind="ExternalInput")
    gm0_t = nc.dram_tensor("gm0", [128, 1], F32, kind="ExternalInput")
    gm33_t = nc.dram_tensor("gm33", [128, 1], F32, kind="ExternalInput")

    out_x = nc.dram_tensor("out_x", [128, 2, NV], F16, kind="ExternalOutput")
    out_y = nc.dram_tensor("out_y", [128, 2, NV], F16, kind="ExternalOutput")
    cc_in = nc.dram_tensor("cc_in", [128, HEADS, 2, 32], F32,
                           kind="Internal")
    cc_out = nc.dram_tensor("cc_out", [128, HEADS, 2, 32], F32,
                            kind="Internal")

    with tile.TileContext(nc) as tc, ExitStack() as ctx:
        wp = ctx.enter_context(tc.tile_pool(name="wp", bufs=1))
        io = ctx.enter_context(tc.tile_pool(name="io", bufs=2))
        hidF = ctx.enter_context(tc.tile_pool(name="hidF", bufs=2))
        hidQ = ctx.enter_context(tc.tile_pool(name="hidQ", bufs=2))
        hidV = ctx.enter_context(tc.tile_pool(name="hidV", bufs=2))
        stk = ctx.enter_context(tc.tile_pool(name="stk", bufs=2))
        sm = ctx.enter_context(tc.tile_pool(name="sm", bufs=1))
        gb = ctx.enter_context(tc.tile_pool(name="gb", bufs=1))
        cvp = ctx.enter_context(tc.tile_pool(name="cvp", bufs=2))
        ot = ctx.enter_context(tc.tile_pool(name="ot", bufs=2))
        cp = ctx.enter_context(tc.tile_pool(name="cp", bufs=12))
        psA = ctx.enter_context(tc.tile_pool(name="psA", bufs=4, space="PSUM"))
        psQ = ctx.enter_context(tc.tile_pool(name="psQ", bufs=2, space="PSUM"))
        psG = ctx.enter_context(tc.tile_pool(name="psG", bufs=1, space="PSUM"))

        # hot weights first, then the first input tiles, then other blobs
        w = {}
        wt = {}
        wt["s"] = wp.tile([128, gsz["s"]], GDT["s"], tag="wg_s",
                          name="wg_s")
        nc.sync.dma_start(wt["s"][:], din["s"].ap())
        xt0 = io.tile([128, 2, 512], F32R, tag="xt")
        nc.sync.dma_start(xt0[:], xin.ap()[:, :, 0:512])
        yt0 = io.tile([128, 2, 512], F32R, tag="yt")
        nc.sync.dma_start(yt0[:], yin.ap()[:, :, 0:512])
        for g in GDT:
            if g == "s":
                continue
            wt[g] = wp.tile([128, gsz[g]], GDT[g], tag=f"wg_{g}",
                            name=f"wg_{g}")
            nc.sync.dma_start(wt[g][:], din[g].ap())
        for name, (g, shp) in WSPEC.items():
            v = wt[g][:, woff[name]:woff[name] + int(np.prod(shp[1:]))]
            if len(shp) == 3:
                v = v.rearrange("p (a b) -> p a b", a=shp[1])
            elif len(shp) == 4:
                v = v.rearrange("p (a b c) -> p a b c", a=shp[1], b=shp[2])
            w[name] = v
        for name, t in (("gm0", gm0_t), ("gm33", gm33_t)):
            tl = wp.tile([128, 1], F32, tag=f"w_{name}", name=f"w_{name}")
            nc.sync.dma_start(tl[:], t.ap())
            w[name] = tl

        # warm only the stage-1 activation tables (the table cache is a
        # small LRU — extra entries thrash it; Lrelu last = most recent)
        warm = sm.tile([128, 12], F32, tag="warm")
        nc.vector.memset(warm[:], 1.0)
        wf16 = sm.tile([128, 4], F16, tag="wf16")
        nc.scalar.copy(warm[:, 3:4], warm[:, 0:1])
        nc.scalar.activation(wf16[:, 0:1], warm[:, 0:1], AF.Gelu,
                             bias=warm[:, 1:2])
        nc.scalar.activation(warm[:, 2:3], warm[:, 0:1], AF.Lrelu,
                             bias=warm[:, 1:2], alpha=LRELU_A)

        gram0 = psG.tile([128, 512], F32, tag="gram0")
        gram1 = psG.tile([128, 512], F32, tag="gram1")
        grams = [gram0, gram1]

        gx = gb.tile([128, 2, ER - 2, WP], F16, tag="gx")
        gy = gb.tile([128, 2, ER - 2, WP], F16, tag="gy")
        nc.scalar.memzero(gx[:])
        nc.scalar.memzero(gy[:])
        TAPS = [(dr, dc) for dr in (-1, 0, 1) for dc in (-1, 0, 1)]

        # v lives in SBUF for its whole life (no DRAM spill): pad columns
        # zeroed once, interiors overwritten tile by tile
        vb_x = gb.tile([128, 2, ER, WP], F16, tag="vb_x")
        vb_y = gb.tile([128, 2, ER, WP], F16, tag="vb_y")
        for vb in (vb_x, vb_y):
            nc.vector.memset(vb[:, :, :, 0:1], 0.0)
            nc.vector.memset(vb[:, :, :, WP - 1:WP], 0.0)

        def conv1_half(vb, gbuf, g0, g1, g):
            """DVE 9-tap fp16 conv1 of one channel half + Gelu evict."""
            nr = g1 - g0
            acc = cvp.tile([128, 6, 128], F16, tag="cacc", name="cacc")
            for i, (dr, dc) in enumerate(TAPS):
                src = vb[:, g, g0 + 1 + dr:g0 + 1 + dr + nr,
                         1 + dc:129 + dc]
                if i == 0:
                    nc.vector.tensor_scalar_mul(acc[:, :nr, :], src,
                                                w["w1c"][:, g, 0:1])
                else:
                    nc.vector.scalar_tensor_tensor(
                        acc[:, :nr, :], src, w["w1c"][:, g, i:i + 1],
                        acc[:, :nr, :], OP.mult, OP.add)
            nc.scalar.activation(gbuf[:, g, g0:g1, 1:129], acc[:, :nr, :],
                                 AF.Gelu, bias=w["b1c"][:, g:g + 1])

        c1q = []

        def push_chunk(gbuf, vb, g0, g1):
            c1q.append(lambda: conv1_half(vb, gbuf, g0, g1, 0))
            c1q.append(lambda: conv1_half(vb, gbuf, g0, g1, 1))

        def drain_one():
            if c1q:
                c1q.pop(0)()

        # ================= stage 1 =================
        vrow = 0
        prev_st = None

        def emit_gram(st, row):
            for h in range(HEADS):
                nc.tensor.matmul(
                    grams[h // 4][:, (h % 4) * 128:(h % 4) * 128 + 128],
                    st[:, h], st[:, h],
                    start=(row == 0), stop=(row == RB - 1),
                    skip_group_check=True)

        def mlp1(srcs, w1T, nk, bias, tag, pool, dt, lo=0, n=512):
            """hidden = lrelu(srcs @ w1T + b); per-half PSUM banks."""
            ht = pool.tile([128, 2, 512], dt, tag=tag)
            for mh in range(2):
                ps = psA.tile([128, 512], F32, tag="psA")
                for k in range(nk):
                    src = srcs[k // 2][:, k % 2, lo:lo + n] if len(srcs) > 1 \
                        else srcs[0][:, k, lo:lo + n]
                    nc.tensor.matmul(ps[:, :n], w1T[:, k, mh, :], src,
                                     start=(k == 0), stop=(k == nk - 1))
                nc.scalar.activation(ht[:, mh, :n], ps[:, :n], AF.Lrelu,
                                     bias=bias[:, mh:mh + 1], alpha=LRELU_A)
            return ht

        for t in range(NT):
            if t == 0:
                xt, yt = xt0, yt0
            else:
                xt = io.tile([128, 2, 512], F32R, tag="xt")
                nc.sync.dma_start(xt[:],
                                  xin.ap()[:, :, t * 512:(t + 1) * 512])
                yt = io.tile([128, 2, 512], F32R, tag="yt")
                nc.sync.dma_start(yt[:],
                                  yin.ap()[:, :, t * 512:(t + 1) * 512])

            # valid-row window within this tile
            e0, e1 = max(2, 4 * t), min(ER - 2, 4 * t + 4)
            lo, n = (e0 - 4 * t) * 128, (e1 - e0) * 128

            fhx = mlp1([xt, yt], w["fxw1T"], 4, w["bfx"], "fhx", hidF, BF16,
                       lo, n)
            fhy = mlp1([xt, yt], w["fyw1T"], 4, w["bfy"], "fhy", hidF, BF16,
                       lo, n)
            qhx = mlp1([xt], w["qw1T"], 2, w["bq"], "qhx", hidQ, BF16, lo, n)
            qhy = mlp1([yt], w["qw1T"], 2, w["bq"], "qhy", hidQ, BF16, lo, n)
            khx = mlp1([fhx], w["kxw1T"], 2, w["bkx"], "khx", hidQ, BF16,
                       0, n)
            khy = mlp1([fhy], w["kyw1T"], 2, w["bky"], "khy", hidQ, BF16,
                       0, n)
            vhx = mlp1([xt], w["vw1T"], 2, w["bv"], "vhx", hidV, F32R)
            vhy = mlp1([yt], w["vw1T"], 2, w["bv"], "vhy", hidV, F32R)

            # v = vhid @ vw2T (ext tokens), evicted straight into SBUF vb
            for nm, vh, vb in (("x", vhx, vb_x), ("y", vhy, vb_y)):
                for g in range(2):
                    ps = psA.tile([128, 512], F32, tag="psA")
                    for k in range(2):
                        nc.tensor.matmul(ps[:], w["vw2T"][:, k, g, :],
                                         vh[:, k, :], start=(k == 0),
                                         stop=(k == 1))
                    nc.vector.tensor_copy(
                        vb[:, g, 4 * t:4 * t + 4, 1:129],
                        ps.rearrange("p (r c) -> p r c", c=128))

            # token-major QK L2 + Gram per valid image row; st evicts on
            # Scalar; Gram lags a row; one conv1 half-thunk per row on DVE
            for e in range(e0, e1):
                off = (e - e0) * 128
                st = stk.tile([128, HEADS, 4, DH], BF16, tag="st")
                for half in range(2):
                    ps = psQ.tile([128, 2, 256], F32, tag="psQ")
                    for s2 in range(2):
                        hh, w2T = ((khy, "kw2T"), (qhx, "qw2T"),
                                   (khx, "kw2T"), (qhy, "qw2T"))[half * 2 + s2]
                        for k in range(2):
                            nc.tensor.matmul(ps[:, s2, :],
                                             hh[:, k, off:off + 128],
                                             w[w2T][:, k, :], start=(k == 0),
                                             stop=(k == 1))
                    nc.scalar.copy(
                        st[:, :, half * 2:half * 2 + 2, :],
                        ps.rearrange("p s (h d) -> p h s d", h=HEADS))
                if prev_st is not None:
                    emit_gram(*prev_st)
                prev_st = (st, vrow)
                vrow += 1
                if t < NT - 1:
                    drain_one()

            for g0, g1, after in C1CHUNKS:
                if after == t:
                    push_chunk(gx, vb_x, g0, g1)
                    push_chunk(gy, vb_y, g0, g1)

        emit_gram(*prev_st)

        # ================= Gram -> AllReduce =================
        # compact payload: per head, slot 0 = the four diagonal norm
        # blocks (stream s at partitions 32s), slot 1 = the off-diagonal
        # logits block (ky^Tqx for x heads at partitions 0:32, kx^Tqy for
        # y heads at 64:96). 64KB instead of the full 512KB Gram.
        gsb = sm.tile([128, 8, 2, 32], F32, tag="gsb")
        nc.scalar.memzero(gsb[:])
        for jt, gr in enumerate(grams):
            grv = gr.rearrange("p (m e) -> p m e", e=128)
            for s4 in range(4):
                nc.scalar.copy(
                    gsb[32 * s4:32 * s4 + 32, 4 * jt:4 * jt + 4, 0, :],
                    grv[32 * s4:32 * s4 + 32, :, 32 * s4:32 * s4 + 32])
            nc.scalar.copy(gsb[0:32, 4 * jt:4 * jt + 4, 1, :],
                           grv[0:32, :, 32:64])
            nc.scalar.copy(gsb[64:96, 4 * jt:4 * jt + 4, 1, :],
                           grv[64:96, :, 96:128])
        nc.sync.dma_start(cc_in.ap(), gsb[:])
        nc.gpsimd.collective_compute(
            "AllReduce", OP.add,
            ins=[cc_in.ap()], outs=[cc_out.ap()],
            replica_groups=[[0, 1, 2, 3], [4, 5, 6, 7]])

        # last conv1 chunk + boundary masking overlap the collective
        for g0, g1, after in C1CHUNKS:
            if after is None:
                push_chunk(gx, vb_x, g0, g1)
                push_chunk(gy, vb_y, g0, g1)
        while c1q:
            drain_one()
        for gbuf in (gx, gy):
            nc.vector.tensor_scalar_mul(gbuf[:, :, 0, :], gbuf[:, :, 0, :],
                                        w["gm0"][:])
            nc.vector.tensor_scalar_mul(gbuf[:, :, ER - 3, :],
                                        gbuf[:, :, ER - 3, :], w["gm33"][:])

        # softmax input DMAs (issued early; they wait on the collective)
        # dg index: 0=(x,g0) 1=(x,g1) 2=(y,g0) 3=(y,g1)
        # (P0 = off-diag partition base, K0/Q0 = k/q diag partition bases)
        s_t = sm.tile([128, 4, DH], F32, tag="s_t")
        db = sm.tile([128, 4, 2, DH], F32, tag="db")
        for dg in range(4):
            g = dg % 2
            P0 = 0 if dg < 2 else 64
            K0, Q0 = P0, P0 + 32
            def _blk(p0, slot):
                return cc_out.ap()[p0:p0 + 32, 4 * g:4 * g + 4, slot, :] \
                    .rearrange("d j e -> j d e")
            nc.sync.dma_start(s_t[:, dg, :], _blk(P0, 1))
            nc.sync.dma_start(db[:, dg, 0, :], _blk(K0, 0))
            nc.sync.dma_start(db[:, dg, 1, :], _blk(Q0, 0))

        # ========== final phase: conv2 groups (PE, evicted to SBUF) =======
        pairs = [(d, tt) for tt in range(8) for d in ("x", "y")]
        FIN = {"x": (vb_x, gx, "obx", out_x), "y": (vb_y, gy, "oby", out_y)}
        cparts = {}

        def emit_conv2(i):
            d, tt = pairs[i]
            vsp, gbuf, ob, o_dram = FIN[d]
            cpt = cp.tile([128, 2, 512], F16, tag="cpart", name="cpart")
            for mo in range(2):
                ps = psA.tile([128, 512], F32, tag="psA", name="finA")
                for i9, (dr, dc) in enumerate(TAPS):
                    src = gbuf[:, mo, 4 * tt + 1 + dr:4 * tt + 5 + dr,
                               1 + dc:129 + dc]
                    nc.tensor.matmul(ps[:], w["dw2"][:, mo, i9, :],
                                     src, start=(i9 == 0), stop=(i9 == 8),
                                     skip_group_check=True)
                nc.scalar.activation(cpt[:, mo, :], ps[:], AF.Identity,
                                     bias=w[ob][:, mo:mo + 1])
            cparts[i] = cpt

        projb_i = [0]

        def proj_bank():
            j = projb_i[0] % 4
            projb_i[0] += 1
            if j < 2:
                return psQ.tile([128, 2, 256], F32, tag="psQ",
                                name="finQ").rearrange("p a b -> p (a b)")
            return grams[j - 2][:]

        def emit_proj(i):
            d, tt = pairs[i]
            vb, _, _, o_dram = FIN[d]
            m1t = m1ts[d]
            cpt = cparts.pop(i)
            o_t = ot.tile([128, 2, 4, 128], F16, tag="o_t")
            for mo in range(2):
                ps = proj_bank()
                for ke in range(2):
                    rhs = vb[:, ke, 4 * tt + 2:4 * tt + 6, 1:129]
                    nc.tensor.matmul(ps, m1t[:, ke, mo, :], rhs,
                                     start=(ke == 0), stop=False,
                                     skip_group_check=True)
                nc.tensor.matmul(ps, w["eye128"][:], cpt[:, mo, :],
                                 start=False, stop=True,
                                 skip_group_check=True)
                nc.scalar.copy(o_t[:, mo, :, :],
                               ps.rearrange("p (r c) -> p r c", c=128))
            nc.sync.dma_start(
                o_dram.ap()[:, :, tt * 512:(tt + 1) * 512],
                o_t.rearrange("p a r c -> p a (r c)"))

        # ten conv2 groups precede the softmax on the PE queue: they run
        # through the AllReduce window (psA rotation, 2 groups in flight)
        emitted = 0
        while emitted < 12:
            emit_conv2(emitted)
            emitted += 1

        # ====== softmax + BD + fused proj matrices (x and y batched) ======
        dbv = db.rearrange("p a b d -> p (a b) d")
        nc.vector.tensor_tensor(dbv[:], dbv[:], w["eye8"][:], OP.mult)
        nkq = sm.tile([128, 4, 2], F32, tag="nkq")
        nc.vector.tensor_reduce(nkq.rearrange("p a b -> p (a b)")[:],
                                dbv[:], mybir.AxisListType.X, OP.add)
        inv = sm.tile([128, 4, 2], F32, tag="inv")
        nc.scalar.sqrt(inv[:], nkq[:])
        nc.vector.reciprocal(inv[:], inv[:])
        ks = sm.tile([128, 4], F32, tag="ks")
        nc.vector.tensor_tensor(ks[:], inv[:, :, 0], w["rxy_exp"][:], OP.mult)
        # qs[p, dg, j] = 1/||q_(head group(p), j)|| broadcast via blk128
        ei = sm.tile([128, 4, DH], F32R, tag="ei")
        for dg in range(4):
            nc.vector.tensor_scalar_mul(ei[:, dg, :], w["eye32r"][:],
                                        inv[:, dg, 1:2])
        pq = psQ.tile([128, 4, DH], F32, tag="psQ")
        nc.tensor.matmul(pq.rearrange("p a d -> p (a d)")[:], w["blk128"][:],
                         ei.rearrange("p a d -> p (a d)")[:],
                         start=True, stop=True)
        qks = sm.tile([128, 4, DH], F32, tag="qks")
        for dg in range(4):
            if dg % 2 == 0:
                nc.scalar.activation(qks[:, dg, :], pq[:, dg, :],
                                     AF.Identity, scale=ks[:, dg:dg + 1])
            else:
                nc.vector.tensor_scalar_mul(qks[:, dg, :], pq[:, dg, :],
                                            ks[:, dg:dg + 1])
        lg = sm.tile([128, 4, DH], F32, tag="lg")
        nc.vector.tensor_tensor(lg[:], s_t[:], qks[:], OP.mult)
        mx = sm.tile([128, 4], F32, tag="mx")
        nc.vector.tensor_reduce(mx[:], lg[:], mybir.AxisListType.X, OP.max)
        nc.vector.tensor_scalar_mul(mx[:], mx[:], -1.0)
        pe_ = sm.tile([128, 4, DH], F32, tag="pe_")
        ssum = sm.tile([128, 4], F32, tag="ssum")
        for dg in range(4):
            nc.scalar.activation(pe_[:, dg, :], lg[:, dg, :], AF.Exp,
                                 bias=mx[:, dg:dg + 1],
                                 accum_out=ssum[:, dg:dg + 1])
        nc.vector.reciprocal(ssum[:], ssum[:])
        at = sm.tile([128, 4, DH], F32, tag="at")
        for dg in range(4):
            if dg % 2 == 0:
                nc.vector.tensor_scalar_mul(at[:, dg, :], pe_[:, dg, :],
                                            ssum[:, dg:dg + 1])
            else:
                nc.scalar.activation(at[:, dg, :], pe_[:, dg, :],
                                     AF.Identity,
                                     scale=ssum[:, dg:dg + 1])
        m1ts = {}
        for d, (dgb, pwT) in {"x": (0, "pxwT"), "y": (2, "pywT")}.items():
            bds = sm.tile([128, 2, 256], F32, tag="bds")
            nc.vector.memset(bds[:], 0.0)
            for g in range(2):
                for j in range(4):
                    h = 4 * g + j
                    dst = bds[j * DH:(j + 1) * DH, g, h * DH:(h + 1) * DH]
                    src = at[j * DH:(j + 1) * DH, dgb + g, :]
                    if j % 2 == 0:
                        nc.vector.tensor_copy(dst, src)
                    else:
                        nc.scalar.copy(dst, src)
            bd = sm.tile([128, 2, 256], F32R, tag="bd")
            nc.vector.tensor_copy(bd[:], bds[:])
            m1t = sm.tile([128, 2, 2, 128], F16, tag=f"m1t_{d}")
            for me in range(2):
                ps = psQ.tile([128, 256], F32, tag="psQ")
                for g in range(2):
                    nc.tensor.matmul(ps[:],
                                     bd[:, g, me * 128:me * 128 + 128],
                                     w[pwT][:, g, :], start=(g == 0),
                                     stop=(g == 1))
                nc.scalar.copy(m1t[:, me, :, :],
                               ps.rearrange("p (a b) -> p a b", a=2))
            m1ts[d] = m1t

        # projection pass: proj pair i, then conv2 of pair i+12
        for i in range(len(pairs)):
            while emitted < min(i + 13, len(pairs)):
                emit_conv2(emitted)
                emitted += 1
            emit_proj(i)

    nc.finalize()
    return nc


# ======================= host side =======================

def _prep_core_input(full, b, h0):
    """(H, W, C) rows [h0-2, h0+34) -> channel-major [128, 2, EN] f32
    (zeros outside the image)."""
    arr = np.zeros((ER, W, C), np.float32)
    r0, r1 = h0 - 2, h0 + RB + 2
    cr0, cr1 = max(r0, 0), min(r1, H)
    arr[cr0 - r0:cr1 - r0] = full[b, cr0:cr1]
    cm = arr.transpose(2, 0, 1).reshape(2, 128, EN)
    return np.ascontiguousarray(cm.transpose(1, 0, 2))


def _cm(v):
    return np.ascontiguousarray(v.reshape(2, 128).T.astype(np.float32))


def _lhsT(wm, nk):
    t = wm.T.reshape(nk, 128, 2, 128)
    return np.ascontiguousarray(t.transpose(1, 0, 2, 3).astype(np.float32))


def _rhsT(wm, dt=np.float32):
    t = wm.T.reshape(2, 128, wm.shape[0])
    return np.ascontiguousarray(t.transpose(1, 0, 2).astype(dt))


def kernel(_trace=False, **inputs):
    inp = {k: np.asarray(v) for k, v in inputs.items()}
    bf = ml_dtypes.bfloat16

    w2c = inp["pe_w2"].reshape(256, 9).astype(np.float32)
    dw2 = np.zeros((128, 2, 9, 128), np.float32)
    for g in range(2):
        for t in range(9):
            dw2[np.arange(128), g, t, np.arange(128)] = \
                w2c[g * 128:(g + 1) * 128, t]

    wvals = {
        "fxw1T": _lhsT(inp["fx_w1"], 4), "fyw1T": _lhsT(inp["fy_w1"], 4),
        "qw1T": _lhsT(inp["q_w1"], 2), "vw1T": _lhsT(inp["v_w1"], 2),
        "kxw1T": _lhsT(inp["k_w1"] @ inp["fx_w2"], 2),
        "kyw1T": _lhsT(inp["k_w1"] @ inp["fy_w2"], 2),
        "vw2T": _lhsT(inp["v_w2"], 2),
        "qw2T": _rhsT(inp["q_w2"], bf), "kw2T": _rhsT(inp["k_w2"], bf),
        "pxwT": _rhsT(inp["px_w"]), "pywT": _rhsT(inp["py_w"]),
        "dw2": dw2.astype(np.float16),
        "eye128": np.eye(128, dtype=np.float16),
        "blk128": np.kron(np.eye(4), np.ones((32, 32))).astype(np.float32),
        "eye32r": np.tile(np.eye(32), (4, 1)).astype(np.float32),
        "eye8": np.ascontiguousarray(
            np.broadcast_to(np.tile(np.eye(32), (4, 1))[:, None, :],
                            (128, 8, 32)).astype(np.float32)),
        "bfx": _cm(inp["fx_b1"]), "bfy": _cm(inp["fy_b1"]),
        "bq": _cm(inp["q_b1"]), "bv": _cm(inp["v_b1"]),
        "bkx": _cm(inp["k_w1"] @ inp["fx_b2"] + inp["k_b1"]),
        "bky": _cm(inp["k_w1"] @ inp["fy_b2"] + inp["k_b1"]),
        "obx": _cm(inp["px_b"] + inp["pe_b2"]),
        "oby": _cm(inp["py_b"] + inp["pe_b2"]),
        "w1c": np.ascontiguousarray(
            inp["pe_w1"].reshape(256, 9).reshape(2, 128, 9)
            .transpose(1, 0, 2).astype(np.float32)),
        "b1c": _cm(inp["pe_b1"]),
        "rxy_exp": np.ascontiguousarray(np.concatenate([
            np.repeat(inp["rescale_x"].reshape(2, 4), 32, axis=1).T,
            np.repeat(inp["rescale_y"].reshape(2, 4), 32, axis=1).T,
        ], axis=1).astype(np.float32)),
    }
    npdt = {"s": np.float32, "r": np.float32, "f": np.float32, "b": bf,
            "h": np.float16}
    blobs = {}
    for g, dt in npdt.items():
        parts = [wvals[n].reshape(128, -1).astype(dt)
                 for n, (gg, _) in WSPEC.items() if gg == g]
        blobs[g] = np.ascontiguousarray(np.concatenate(parts, axis=1))

    in_maps = []
    for r in range(8):
        b, h0 = r // 4, (r % 4) * RB
        m = {f"wg_{g}": blobs[g] for g in npdt}
        m["xin"] = _prep_core_input(inp["x_in"], b, h0)
        m["yin"] = _prep_core_input(inp["y_in"], b, h0)
        m["gm0"] = np.full((128, 1), 0.0 if h0 == 0 else 1.0, np.float32)
        m["gm33"] = np.full((128, 1), 0.0 if h0 + RB == H else 1.0,
                            np.float32)
        in_maps.append(m)

    if "nc" not in _CACHED:
        _CACHED["nc"] = _nc_build()
    res = run_bass_kernel_spmd(_CACHED["nc"], in_maps,
                               core_ids=list(range(8)), trace=_trace)
    _CACHED["last_result"] = res

    out_x = np.empty((B, H, W, C), np.float32)
    out_y = np.empty((B, H, W, C), np.float32)
    for r in range(8):
        b, h0 = r // 4, (r % 4) * RB
        for name, dst in (("out_x", out_x), ("out_y", out_y)):
            a = res.results[r][name].astype(np.float32)
            a = a.reshape(128, 2, RB, W)
            dst[b, h0:h0 + RB] = a.transpose(2, 3, 1, 0).reshape(RB, W, C)
    return out_x, out_y


# revision 35
# speedup vs baseline: 1.0327x; 1.0131x over previous
"""DMSA (dual-modal channel cross-attention) Trainium2 kernel — v5.

Sharding: 8 cores = 2 batches x 4 bands of 32 image rows. Each core
computes its band fully; the channel attention's per-head Gram matrices
(contraction over all n = h*w tokens, with l2-normalization folded in
via the Gram diagonal) are summed with one AllReduce per 4-core group.

Device layout: channel-major activations [128 partitions, 2 channel
halves, tokens]. Stage-1 runs on an unpadded 36x128 ext-row grid
(9 tiles x 512 tokens); v is spilled in fp16 to a width-padded 36x130
DRAM grid (pad columns kept zero inside the spill tile) so both 3x3
depthwise convs read taps as plain offset views.

Engines: big matmuls f32r; q/k path bf16; v/conv path fp16. conv1 runs
on DVE in half-chunk thunks drained one per image row so PSUM evicts
never queue behind it. conv2 runs as 9 diagonal PE matmuls per output
group, evicted to SBUF (bias folded) so ten groups precede the softmax
on the PE queue and cover the AllReduce window; the projection pass
then adds on top via DVE. Weights arrive in 5 blob DMAs.
"""
import numpy as np
import ml_dtypes
from contextlib import ExitStack

import concourse.bass as bass
import concourse.tile as tile
import concourse.mybir as mybir
from concourse import bacc
from concourse.bass_utils import run_bass_kernel_spmd

F32 = mybir.dt.float32
F32R = mybir.dt.float32r
BF16 = mybir.dt.bfloat16
F16 = mybir.dt.float16
AF = mybir.ActivationFunctionType
OP = mybir.AluOpType

B, H, W, C = 2, 128, 128, 256
HEADS, DH = 8, 32
RB = 32             # image rows per core
ER = RB + 4         # ext rows
WP = W + 2          # padded width (conv grid)
GN = ER * WP        # padded tokens (v spill grid) = 4680
EN = ER * W         # unpadded ext tokens (stage-1 grid) = 4608
NV = RB * W         # valid tokens = 4096
NT = 9              # stage-1 tiles (4 ext rows each)
LRELU_A = 0.01
# conv1 chunk g-row ranges and the stage-1 tile after which each may run
C1CHUNKS = [(0, 6, 1), (6, 12, 3), (12, 18, 4), (18, 24, 6), (24, 30, 7),
            (30, 34, None)]  # None -> after the collective

# weight blobs: name -> (group, shape); group s = hot f32r (tile-0 deps)
WSPEC = {
    "fxw1T": ("s", [128, 4, 2, 128]), "fyw1T": ("s", [128, 4, 2, 128]),
    "qw1T": ("s", [128, 2, 2, 128]), "vw1T": ("s", [128, 2, 2, 128]),
    "vw2T": ("b", [128, 2, 2, 128]), "pxwT": ("r", [128, 2, 256]),
    "pywT": ("r", [128, 2, 256]), "blk128": ("r", [128, 128]),
    "eye32r": ("f", [128, 32]), "eye8": ("f", [128, 8, 32]),
    "bfx": ("f", [128, 2]), "bfy": ("f", [128, 2]), "bq": ("f", [128, 2]),
    "bkx": ("f", [128, 2]), "bky": ("f", [128, 2]), "bv": ("f", [128, 2]),
    "obx": ("f", [128, 2]), "oby": ("f", [128, 2]), "b1c": ("f", [128, 2]),
    "rxy_exp": ("f", [128, 4]), "w1c": ("f", [128, 2, 9]),
    "qw2T": ("b", [128, 2, 256]), "kw2T": ("b", [128, 2, 256]),
    "kxw1T": ("b", [128, 2, 2, 128]), "kyw1T": ("b", [128, 2, 2, 128]),
    "dw2": ("h", [128, 2, 9, 128]), "eye128": ("h", [128, 128]),
}
GDT = {"s": F32R, "r": F32R, "f": F32, "b": BF16, "h": F16}

_CACHED = {}


def _nc_build():
    nc = bacc.Bacc(num_devices=8)

    gsz = {g: 0 for g in GDT}
    woff = {}
    for name, (g, shp) in WSPEC.items():
        n = int(np.prod(shp[1:]))
        woff[name] = gsz[g]
        gsz[g] += n

    din = {}
    for g in GDT:
        din[g] = nc.dram_tensor(f"wg_{g}", [128, gsz[g]], GDT[g],
                                kind="ExternalInput")
    xin = nc.dram_tensor("xin", [128, 2, EN], F32R, kind="ExternalInput")
    yin = nc.dram_tensor("yin", [128, 2, EN], F32R, kind="ExternalInput")
    gm0_t = nc.dram_tensor("gm0", [128, 1], F32, kind="ExternalInput")
    gm33_t = nc.dram_tensor("gm33", [128, 1], F32, kind="ExternalInput")

    out_x = nc.dram_tensor("out_x", [128, 2, NV], F16, kind="ExternalOutput")
    out_y = nc.dram_tensor("out_y", [128, 2, NV], F16, kind="ExternalOutput")
    cc_in = nc.dram_tensor("cc_in", [128, HEADS, 2, 32], F32,
                           kind="Internal")
    cc_out = nc.dram_tensor("cc_out", [128, HEADS, 2, 32], F32,
                            kind="Internal")

    with tile.TileContext(nc) as tc, ExitStack() as ctx:
        wp = ctx.enter_context(tc.tile_pool(name="wp", bufs=1))
        io = ctx.enter_context(tc.tile_pool(name="io", bufs=2))
        hidF = ctx.enter_context(tc.tile_pool(name="hidF", bufs=2))
        hidQ = ctx.enter_context(tc.tile_pool(name="hidQ", bufs=2))
        hidV = ctx.enter_context(tc.tile_pool(name="hidV", bufs=2))
        stk = ctx.enter_context(tc.tile_pool(name="stk", bufs=2))
        sm = ctx.enter_context(tc.tile_pool(name="sm", bufs=1))
        gb = ctx.enter_context(tc.tile_pool(name="gb", bufs=1))
        cvp = ctx.enter_context(tc.tile_pool(name="cvp", bufs=2))
        ot = ctx.enter_context(tc.tile_pool(name="ot", bufs=2))
        cp = ctx.enter_context(tc.tile_pool(name="cp", bufs=12))
        psA = ctx.enter_context(tc.tile_pool(name="psA", bufs=4, space="PSUM"))
        psQ = ctx.enter_context(tc.tile_pool(name="psQ", bufs=2, space="PSUM"))
        psG = ctx.enter_context(tc.tile_pool(name="psG", bufs=1, space="PSUM"))

        # hot weights first, then the first input tiles, then other blobs
        w = {}
        wt = {}
        wt["s"] = wp.tile([128, gsz["s"]], GDT["s"], tag="wg_s",
                          name="wg_s")
        nc.sync.dma_start(wt["s"][:], din["s"].ap())
        xt0 = io.tile([128, 2, 512], F32R, tag="xt")
        nc.sync.dma_start(xt0[:], xin.ap()[:, :, 0:512])
        yt0 = io.tile([128, 2, 512], F32R, tag="yt")
        nc.sync.dma_start(yt0[:], yin.ap()[:, :, 0:512])
        for g in GDT:
            if g == "s":
                continue
            wt[g] = wp.tile([128, gsz[g]], GDT[g], tag=f"wg_{g}",
                            name=f"wg_{g}")
            nc.sync.dma_start(wt[g][:], din[g].ap())
        for name, (g, shp) in WSPEC.items():
            v = wt[g][:, woff[name]:woff[name] + int(np.prod(shp[1:]))]
            if len(shp) == 3:
                v = v.rearrange("p (a b) -> p a b", a=shp[1])
            elif len(shp) == 4:
                v = v.rearrange("p (a b c) -> p a b c", a=shp[1], b=shp[2])
            w[name] = v
        for name, t in (("gm0", gm0_t), ("gm33", gm33_t)):
            tl = wp.tile([128, 1], F32, tag=f"w_{name}", name=f"w_{name}")
            nc.sync.dma_start(tl[:], t.ap())
            w[name] = tl

        # warm only the stage-1 activation tables (the table cache is a
        # small LRU — extra entries thrash it; Lrelu last = most recent)
        warm = sm.tile([128, 12], F32, tag="warm")
        nc.vector.memset(warm[:], 1.0)
        wf16 = sm.tile([128, 4], F16, tag="wf16")
        nc.scalar.copy(warm[:, 3:4], warm[:, 0:1])
        nc.scalar.activation(wf16[:, 0:1], warm[:, 0:1], AF.Gelu,
                             bias=warm[:, 1:2])
        nc.scalar.activation(warm[:, 2:3], warm[:, 0:1], AF.Lrelu,
                             bias=warm[:, 1:2], alpha=LRELU_A)

        gram0 = psG.tile([128, 512], F32, tag="gram0")
        gram1 = psG.tile([128, 512], F32, tag="gram1")
        grams = [gram0, gram1]

        gx = gb.tile([128, 2, ER - 2, WP], F16, tag="gx")
        gy = gb.tile([128, 2, ER - 2, WP], F16, tag="gy")
        nc.scalar.memzero(gx[:])
        nc.scalar.memzero(gy[:])
        TAPS = [(dr, dc) for dr in (-1, 0, 1) for dc in (-1, 0, 1)]

        # v lives in SBUF for its whole life (no DRAM spill): pad columns
        # zeroed once, interiors overwritten tile by tile
        vb_x = gb.tile([128, 2, ER, WP], F16, tag="vb_x")
        vb_y = gb.tile([128, 2, ER, WP], F16, tag="vb_y")
        for vb in (vb_x, vb_y):
            nc.vector.memset(vb[:, :, :, 0:1], 0.0)
            nc.vector.memset(vb[:, :, :, WP - 1:WP], 0.0)

        def conv1_half(vb, gbuf, g0, g1, g):
            """DVE 9-tap fp16 conv1 of one channel half + Gelu evict."""
            nr = g1 - g0
            acc = cvp.tile([128, 6, 128], F16, tag="cacc", name="cacc")
            for i, (dr, dc) in enumerate(TAPS):
                src = vb[:, g, g0 + 1 + dr:g0 + 1 + dr + nr,
                         1 + dc:129 + dc]
                if i == 0:
                    nc.vector.tensor_scalar_mul(acc[:, :nr, :], src,
                                                w["w1c"][:, g, 0:1])
                else:
                    nc.vector.scalar_tensor_tensor(
                        acc[:, :nr, :], src, w["w1c"][:, g, i:i + 1],
                        acc[:, :nr, :], OP.mult, OP.add)
            nc.scalar.activation(gbuf[:, g, g0:g1, 1:129], acc[:, :nr, :],
                                 AF.Gelu, bias=w["b1c"][:, g:g + 1])

        c1q = []

        def push_chunk(gbuf, vb, g0, g1):
            c1q.append(lambda: conv1_half(vb, gbuf, g0, g1, 0))
            c1q.append(lambda: conv1_half(vb, gbuf, g0, g1, 1))

        def drain_one():
            if c1q:
                c1q.pop(0)()

        # ================= stage 1 =================
        vrow = 0
        prev_st = None

        def emit_gram(st, row):
            for h in range(HEADS):
                nc.tensor.matmul(
                    grams[h // 4][:, (h % 4) * 128:(h % 4) * 128 + 128],
                    st[:, h], st[:, h],
                    start=(row == 0), stop=(row == RB - 1),
                    skip_group_check=True)

        def mlp1(srcs, w1T, nk, bias, tag, pool, dt, lo=0, n=512):
            """hidden = lrelu(srcs @ w1T + b); per-half PSUM banks."""
            ht = pool.tile([128, 2, 512], dt, tag=tag)
            for mh in range(2):
                ps = psA.tile([128, 512], F32, tag="psA")
                for k in range(nk):
                    src = srcs[k // 2][:, k % 2, lo:lo + n] if len(srcs) > 1 \
                        else srcs[0][:, k, lo:lo + n]
                    nc.tensor.matmul(ps[:, :n], w1T[:, k, mh, :], src,
                                     start=(k == 0), stop=(k == nk - 1))
                nc.scalar.activation(ht[:, mh, :n], ps[:, :n], AF.Lrelu,
                                     bias=bias[:, mh:mh + 1], alpha=LRELU_A)
            return ht

        for t in range(NT):
            if t == 0:
                xt, yt = xt0, yt0
            else:
                xt = io.tile([128, 2, 512], F32R, tag="xt")
                nc.sync.dma_start(xt[:],
                                  xin.ap()[:, :, t * 512:(t + 1) * 512])
                yt = io.tile([128, 2, 512], F32R, tag="yt")
                nc.sync.dma_start(yt[:],
                                  yin.ap()[:, :, t * 512:(t + 1) * 512])

            # valid-row window within this tile
            e0, e1 = max(2, 4 * t), min(ER - 2, 4 * t + 4)
            lo, n = (e0 - 4 * t) * 128, (e1 - e0) * 128

            fhx = mlp1([xt, yt], w["fxw1T"], 4, w["bfx"], "fhx", hidF, BF16,
                       lo, n)
            fhy = mlp1([xt, yt], w["fyw1T"], 4, w["bfy"], "fhy", hidF, BF16,
                       lo, n)
            qhx = mlp1([xt], w["qw1T"], 2, w["bq"], "qhx", hidQ, BF16, lo, n)
            qhy = mlp1([yt], w["qw1T"], 2, w["bq"], "qhy", hidQ, BF16, lo, n)
            khx = mlp1([fhx], w["kxw1T"], 2, w["bkx"], "khx", hidQ, BF16,
                       0, n)
            khy = mlp1([fhy], w["kyw1T"], 2, w["bky"], "khy", hidQ, BF16,
                       0, n)
            vhx = mlp1([xt], w["vw1T"], 2, w["bv"], "vhx", hidV, BF16)
            vhy = mlp1([yt], w["vw1T"], 2, w["bv"], "vhy", hidV, BF16)

            # v = vhid @ vw2T (ext tokens), evicted straight into SBUF vb
            for nm, vh, vb in (("x", vhx, vb_x), ("y", vhy, vb_y)):
                for g in range(2):
                    ps = psA.tile([128, 512], F32, tag="psA")
                    for k in range(2):
                        nc.tensor.matmul(ps[:], w["vw2T"][:, k, g, :],
                                         vh[:, k, :], start=(k == 0),
                                         stop=(k == 1))
                    nc.vector.tensor_copy(
                        vb[:, g, 4 * t:4 * t + 4, 1:129],
                        ps.rearrange("p (r c) -> p r c", c=128))

            # token-major QK L2 + Gram per valid image row; st evicts on
            # Scalar; Gram lags a row; one conv1 half-thunk per row on DVE
            for e in range(e0, e1):
                off = (e - e0) * 128
                st = stk.tile([128, HEADS, 4, DH], BF16, tag="st")
                for half in range(2):
                    ps = psQ.tile([128, 2, 256], F32, tag="psQ")
                    for s2 in range(2):
                        hh, w2T = ((khy, "kw2T"), (qhx, "qw2T"),
                                   (khx, "kw2T"), (qhy, "qw2T"))[half * 2 + s2]
                        for k in range(2):
                            nc.tensor.matmul(ps[:, s2, :],
                                             hh[:, k, off:off + 128],
                                             w[w2T][:, k, :], start=(k == 0),
                                             stop=(k == 1))
                    nc.scalar.copy(
                        st[:, :, half * 2:half * 2 + 2, :],
                        ps.rearrange("p s (h d) -> p h s d", h=HEADS))
                if prev_st is not None:
                    emit_gram(*prev_st)
                prev_st = (st, vrow)
                vrow += 1
                if t < NT - 1:
                    drain_one()

            for g0, g1, after in C1CHUNKS:
                if after == t:
                    push_chunk(gx, vb_x, g0, g1)
                    push_chunk(gy, vb_y, g0, g1)

        emit_gram(*prev_st)

        # ================= Gram -> AllReduce =================
        # compact payload: per head, slot 0 = the four diagonal norm
        # blocks (stream s at partitions 32s), slot 1 = the off-diagonal
        # logits block (ky^Tqx for x heads at partitions 0:32, kx^Tqy for
        # y heads at 64:96). 64KB instead of the full 512KB Gram.
        gsb = sm.tile([128, 8, 2, 32], F32, tag="gsb")
        nc.scalar.memzero(gsb[:])
        for jt, gr in enumerate(grams):
            grv = gr.rearrange("p (m e) -> p m e", e=128)
            for s4 in range(4):
                nc.scalar.copy(
                    gsb[32 * s4:32 * s4 + 32, 4 * jt:4 * jt + 4, 0, :],
                    grv[32 * s4:32 * s4 + 32, :, 32 * s4:32 * s4 + 32])
            nc.scalar.copy(gsb[0:32, 4 * jt:4 * jt + 4, 1, :],
                           grv[0:32, :, 32:64])
            nc.scalar.copy(gsb[64:96, 4 * jt:4 * jt + 4, 1, :],
                           grv[64:96, :, 96:128])
        nc.sync.dma_start(cc_in.ap(), gsb[:])
        nc.gpsimd.collective_compute(
            "AllReduce", OP.add,
            ins=[cc_in.ap()], outs=[cc_out.ap()],
            replica_groups=[[0, 1, 2, 3], [4, 5, 6, 7]])

        # last conv1 chunk + boundary masking overlap the collective
        for g0, g1, after in C1CHUNKS:
            if after is None:
                push_chunk(gx, vb_x, g0, g1)
                push_chunk(gy, vb_y, g0, g1)
        while c1q:
            drain_one()
        for gbuf in (gx, gy):
            nc.vector.tensor_scalar_mul(gbuf[:, :, 0, :], gbuf[:, :, 0, :],
                                        w["gm0"][:])
            nc.vector.tensor_scalar_mul(gbuf[:, :, ER - 3, :],
                                        gbuf[:, :, ER - 3, :], w["gm33"][:])

        # softmax input DMAs (issued early; they wait on the collective)
        # dg index: 0=(x,g0) 1=(x,g1) 2=(y,g0) 3=(y,g1)
        # (P0 = off-diag partition base, K0/Q0 = k/q diag partition bases)
        s_t = sm.tile([128, 4, DH], F32, tag="s_t")
        db = sm.tile([128, 4, 2, DH], F32, tag="db")
        for dg in range(4):
            g = dg % 2
            P0 = 0 if dg < 2 else 64
            K0, Q0 = P0, P0 + 32
            def _blk(p0, slot):
                return cc_out.ap()[p0:p0 + 32, 4 * g:4 * g + 4, slot, :] \
                    .rearrange("d j e -> j d e")
            nc.sync.dma_start(s_t[:, dg, :], _blk(P0, 1))
            nc.sync.dma_start(db[:, dg, 0, :], _blk(K0, 0))
            nc.sync.dma_start(db[:, dg, 1, :], _blk(Q0, 0))

        # ========== final phase: conv2 groups (PE, evicted to SBUF) =======
        pairs = [(d, tt) for tt in range(8) for d in ("x", "y")]
        FIN = {"x": (vb_x, gx, "obx", out_x), "y": (vb_y, gy, "oby", out_y)}
        cparts = {}

        def emit_conv2(i):
            d, tt = pairs[i]
            vsp, gbuf, ob, o_dram = FIN[d]
            cpt = cp.tile([128, 2, 512], F16, tag="cpart", name="cpart")
            for mo in range(2):
                ps = psA.tile([128, 512], F32, tag="psA", name="finA")
                for i9, (dr, dc) in enumerate(TAPS):
                    src = gbuf[:, mo, 4 * tt + 1 + dr:4 * tt + 5 + dr,
                               1 + dc:129 + dc]
                    nc.tensor.matmul(ps[:], w["dw2"][:, mo, i9, :],
                                     src, start=(i9 == 0), stop=(i9 == 8),
                                     skip_group_check=True)
                nc.scalar.activation(cpt[:, mo, :], ps[:], AF.Identity,
                                     bias=w[ob][:, mo:mo + 1])
            cparts[i] = cpt

        projb_i = [0]

        def proj_bank():
            j = projb_i[0] % 4
            projb_i[0] += 1
            if j < 2:
                return psQ.tile([128, 2, 256], F32, tag="psQ",
                                name="finQ").rearrange("p a b -> p (a b)")
            return grams[j - 2][:]

        def emit_proj(i):
            d, tt = pairs[i]
            vb, _, _, o_dram = FIN[d]
            m1t = m1ts[d]
            cpt = cparts.pop(i)
            o_t = ot.tile([128, 2, 4, 128], F16, tag="o_t")
            for mo in range(2):
                ps = proj_bank()
                for ke in range(2):
                    rhs = vb[:, ke, 4 * tt + 2:4 * tt + 6, 1:129]
                    nc.tensor.matmul(ps, m1t[:, ke, mo, :], rhs,
                                     start=(ke == 0), stop=False,
                                     skip_group_check=True)
                nc.tensor.matmul(ps, w["eye128"][:], cpt[:, mo, :],
                                 start=False, stop=True,
                                 skip_group_check=True)
                nc.scalar.copy(o_t[:, mo, :, :],
                               ps.rearrange("p (r c) -> p r c", c=128))
            nc.sync.dma_start(
                o_dram.ap()[:, :, tt * 512:(tt + 1) * 512],
                o_t.rearrange("p a r c -> p a (r c)"))

        # ten conv2 groups precede the softmax on the PE queue: they run
        # through the AllReduce window (psA rotation, 2 groups in flight)
        emitted = 0
        while emitted < 12:
            emit_conv2(emitted)
            emitted += 1

        # ====== softmax + BD + fused proj matrices (x and y batched) ======
        dbv = db.rearrange("p a b d -> p (a b) d")
        nc.vector.tensor_tensor(dbv[:], dbv[:], w["eye8"][:], OP.mult)
        nkq = sm.tile([128, 4, 2], F32, tag="nkq")
        nc.vector.tensor_reduce(nkq.rearrange("p a b -> p (a b)")[:],
                                dbv[:], mybir.AxisListType.X, OP.add)
        inv = sm.tile([128, 4, 2], F32, tag="inv")
        nc.scalar.sqrt(inv[:], nkq[:])
        nc.vector.reciprocal(inv[:], inv[:])
        ks = sm.tile([128, 4], F32, tag="ks")
        nc.vector.tensor_tensor(ks[:], inv[:, :, 0], w["rxy_exp"][:], OP.mult)
        # qs[p, dg, j] = 1/||q_(head group(p), j)|| broadcast via blk128
        ei = sm.tile([128, 4, DH], F32R, tag="ei")
        for dg in range(4):
            nc.vector.tensor_scalar_mul(ei[:, dg, :], w["eye32r"][:],
                                        inv[:, dg, 1:2])
        pq = psQ.tile([128, 4, DH], F32, tag="psQ")
        nc.tensor.matmul(pq.rearrange("p a d -> p (a d)")[:], w["blk128"][:],
                         ei.rearrange("p a d -> p (a d)")[:],
                         start=True, stop=True)
        qks = sm.tile([128, 4, DH], F32, tag="qks")
        for dg in range(4):
            nc.vector.tensor_scalar_mul(qks[:, dg, :], pq[:, dg, :],
                                        ks[:, dg:dg + 1])
        lg = sm.tile([128, 4, DH], F32, tag="lg")
        nc.vector.tensor_tensor(lg[:], s_t[:], qks[:], OP.mult)
        mx = sm.tile([128, 4], F32, tag="mx")
        nc.vector.tensor_reduce(mx[:], lg[:], mybir.AxisListType.X, OP.max)
        nc.vector.tensor_scalar_mul(mx[:], mx[:], -1.0)
        pe_ = sm.tile([128, 4, DH], F32, tag="pe_")
        ssum = sm.tile([128, 4], F32, tag="ssum")
        for dg in range(4):
            nc.scalar.activation(pe_[:, dg, :], lg[:, dg, :], AF.Exp,
                                 bias=mx[:, dg:dg + 1],
                                 accum_out=ssum[:, dg:dg + 1])
        nc.vector.reciprocal(ssum[:], ssum[:])
        at = sm.tile([128, 4, DH], F32, tag="at")
        for dg in range(4):
            nc.vector.tensor_scalar_mul(at[:, dg, :], pe_[:, dg, :],
                                        ssum[:, dg:dg + 1])
        m1ts = {}
        for d, (dgb, pwT) in {"x": (0, "pxwT"), "y": (2, "pywT")}.items():
            bds = sm.tile([128, 2, 256], F32, tag="bds")
            nc.vector.memset(bds[:], 0.0)
            for g in range(2):
                for j in range(4):
                    h = 4 * g + j
                    dst = bds[j * DH:(j + 1) * DH, g, h * DH:(h + 1) * DH]
                    src = at[j * DH:(j + 1) * DH, dgb + g, :]
                    if j % 2 == 0:
                        nc.vector.tensor_copy(dst, src)
                    else:
                        nc.scalar.copy(dst, src)
            bd = sm.tile([128, 2, 256], F32R, tag="bd")
            nc.vector.tensor_copy(bd[:], bds[:])
            m1t = sm.tile([128, 2, 2, 128], F16, tag=f"m1t_{d}")
            for me in range(2):
                ps = psQ.tile([128, 256], F32, tag="psQ")
                for g in range(2):
                    nc.tensor.matmul(ps[:],
                                     bd[:, g, me * 128:me * 128 + 128],
                                     w[pwT][:, g, :], start=(g == 0),
                                     stop=(g == 1))
                nc.scalar.copy(m1t[:, me, :, :],
                               ps.rearrange("p (a b) -> p a b", a=2))
            m1ts[d] = m1t

        # projection pass: proj pair i, then conv2 of pair i+12
        for i in range(len(pairs)):
            while emitted < min(i + 13, len(pairs)):
                emit_conv2(emitted)
                emitted += 1
            emit_proj(i)

    nc.finalize()
    return nc


# ======================= host side =======================

def _prep_core_input(full, b, h0):
    """(H, W, C) rows [h0-2, h0+34) -> channel-major [128, 2, EN] f32
    (zeros outside the image)."""
    arr = np.zeros((ER, W, C), np.float32)
    r0, r1 = h0 - 2, h0 + RB + 2
    cr0, cr1 = max(r0, 0), min(r1, H)
    arr[cr0 - r0:cr1 - r0] = full[b, cr0:cr1]
    cm = arr.transpose(2, 0, 1).reshape(2, 128, EN)
    return np.ascontiguousarray(cm.transpose(1, 0, 2))


def _cm(v):
    return np.ascontiguousarray(v.reshape(2, 128).T.astype(np.float32))


def _lhsT(wm, nk):
    t = wm.T.reshape(nk, 128, 2, 128)
    return np.ascontiguousarray(t.transpose(1, 0, 2, 3).astype(np.float32))


def _rhsT(wm, dt=np.float32):
    t = wm.T.reshape(2, 128, wm.shape[0])
    return np.ascontiguousarray(t.transpose(1, 0, 2).astype(dt))


def kernel(_trace=False, **inputs):
    inp = {k: np.asarray(v) for k, v in inputs.items()}
    bf = ml_dtypes.bfloat16

    w2c = inp["pe_w2"].reshape(256, 9).astype(np.float32)
    dw2 = np.zeros((128, 2, 9, 128), np.float32)
    for g in range(2):
        for t in range(9):
            dw2[np.arange(128), g, t, np.arange(128)] = \
                w2c[g * 128:(g + 1) * 128, t]

    wvals = {
        "fxw1T": _lhsT(inp["fx_w1"], 4), "fyw1T": _lhsT(inp["fy_w1"], 4),
        "qw1T": _lhsT(inp["q_w1"], 2), "vw1T": _lhsT(inp["v_w1"], 2),
        "kxw1T": _lhsT(inp["k_w1"] @ inp["fx_w2"], 2),
        "kyw1T": _lhsT(inp["k_w1"] @ inp["fy_w2"], 2),
        "vw2T": _lhsT(inp["v_w2"], 2),
        "qw2T": _rhsT(inp["q_w2"], bf), "kw2T": _rhsT(inp["k_w2"], bf),
        "pxwT": _rhsT(inp["px_w"]), "pywT": _rhsT(inp["py_w"]),
        "dw2": dw2.astype(np.float16),
        "eye128": np.eye(128, dtype=np.float16),
        "blk128": np.kron(np.eye(4), np.ones((32, 32))).astype(np.float32),
        "eye32r": np.tile(np.eye(32), (4, 1)).astype(np.float32),
        "eye8": np.ascontiguousarray(
            np.broadcast_to(np.tile(np.eye(32), (4, 1))[:, None, :],
                            (128, 8, 32)).astype(np.float32)),
        "bfx": _cm(inp["fx_b1"]), "bfy": _cm(inp["fy_b1"]),
        "bq": _cm(inp["q_b1"]), "bv": _cm(inp["v_b1"]),
        "bkx": _cm(inp["k_w1"] @ inp["fx_b2"] + inp["k_b1"]),
        "bky": _cm(inp["k_w1"] @ inp["fy_b2"] + inp["k_b1"]),
        "obx": _cm(inp["px_b"] + inp["pe_b2"]),
        "oby": _cm(inp["py_b"] + inp["pe_b2"]),
        "w1c": np.ascontiguousarray(
            inp["pe_w1"].reshape(256, 9).reshape(2, 128, 9)
            .transpose(1, 0, 2).astype(np.float32)),
        "b1c": _cm(inp["pe_b1"]),
        "rxy_exp": np.ascontiguousarray(np.concatenate([
            np.repeat(inp["rescale_x"].reshape(2, 4), 32, axis=1).T,
            np.repeat(inp["rescale_y"].reshape(2, 4), 32, axis=1).T,
        ], axis=1).astype(np.float32)),
    }
    npdt = {"s": np.float32, "r": np.float32, "f": np.float32, "b": bf,
            "h": np.float16}
    blobs = {}
    for g, dt in npdt.items():
        parts = [wvals[n].reshape(128, -1).astype(dt)
                 for n, (gg, _) in WSPEC.items() if gg == g]
        blobs[g] = np.ascontiguousarray(np.concatenate(parts, axis=1))

    in_maps = []
    for r in range(8):
        b, h0 = r // 4, (r % 4) * RB
        m = {f"wg_{g}": blobs[g] for g in npdt}
        m["xin"] = _prep_core_input(inp["x_in"], b, h0)
        m["yin"] = _prep_core_input(inp["y_in"], b, h0)
        m["gm0"] = np.full((128, 1), 0.0 if h0 == 0 else 1.0, np.float32)
        m["gm33"] = np.full((128, 1), 0.0 if h0 + RB == H else 1.0,
                            np.float32)
        in_maps.append(m)

    if "nc" not in _CACHED:
        _CACHED["nc"] = _nc_build()
    res = run_bass_kernel_spmd(_CACHED["nc"], in_maps,
                               core_ids=list(range(8)), trace=_trace)
    _CACHED["last_result"] = res

    out_x = np.empty((B, H, W, C), np.float32)
    out_y = np.empty((B, H, W, C), np.float32)
    for r in range(8):
        b, h0 = r // 4, (r % 4) * RB
        for name, dst in (("out_x", out_x), ("out_y", out_y)):
            a = res.results[r][name].astype(np.float32)
            a = a.reshape(128, 2, RB, W)
            dst[b, h0:h0 + RB] = a.transpose(2, 3, 1, 0).reshape(RB, W, C)
    return out_x, out_y
